# revision 9
# baseline (speedup 1.0000x reference)
"""Transformer decoder layer (self-attn + cross-attn + FFN, pre-output LNs)
on 8 Trainium2 NeuronCores, sequence-parallel with zero collectives.

Sharding: core c -> batch b = c//4, causal-balanced chunk pair (j, 7-j) of
256 tokens each (j = c%4), so every core owns 512 query tokens with equal
total causal attention area. Weights are replicated; K/V projections are
recomputed per core. All per-core differences are expressed through input
DATA (token reordering + additive exp-bias masks), so a single SPMD program
serves all 8 cores.

Layout: activations are kept feature-major [d_partition, token_free] so
every matmul contracts along SBUF partitions with weights as the stationary
operand. Scores are computed transposed (S^T = [s, q]) which makes softmax
need no cross-partition reduction: exp on ScalarE (scores are O(0.5), so no
max subtraction), denominator via an extra ones-column appended to V.
Matmuls run in bf16 with fp32 PSUM accumulation; the residual stream and
layernorms stay fp32 (LN stats/broadcasts via full-rate float32r matmuls).
"""

import sys

if "/opt/trn_rl_repo" not in sys.path:
    sys.path.insert(0, "/opt/trn_rl_repo")

from contextlib import ExitStack

import numpy as np
import ml_dtypes

import concourse.bass as bass
import concourse.bacc as bacc
import concourse.tile as tile
import concourse.mybir as mybir
from concourse.bass_utils import run_bass_kernel_spmd
from concourse.masks import make_identity

F32 = mybir.dt.float32
F32R = mybir.dt.float32r
BF16 = mybir.dt.bfloat16
AF = mybir.ActivationFunctionType
ALU = mybir.AluOpType

D = 1024
H = 16
DK = 64
DFF = 4096
B = 2
T = 2048
N_CORES = 8
CHUNK = 256
TQ = 512          # query tokens per core
KV = 2048         # padded kv layout length (self), enc length (cross)
FT = D // 128     # 8 f-tiles
HT = DFF // 128   # 32 ffn tiles
NEG = -50.0       # additive pre-exp mask (exp(-50) ~ 2e-22)

# self-attn static s-tile schedules (128-token tiles over the kv layout
# [A(256) | B(256) | rest... | pad]):
SA_TILES = [0, 1, 4, 5, 6, 7, 8, 9]   # chunk A: own diag + prior window
SB_TILES = list(range(16))            # chunk B: everything real
DIAG_A = {0: 0, 1: 256}               # s-tile -> dmask col offset
DIAG_B = {2: 0, 3: 256}

_BUILT = None


def _build():
    nc = bacc.Bacc("TRN2", target_bir_lowering=False, debug=False,
                   num_devices=N_CORES)

    def din(name, shape, dt):
        return nc.dram_tensor(name, shape, dt, kind="ExternalInput").ap()

    xq_d = din("xq", [128, FT, TQ], BF16)
    xres_d = din("xres", [128, FT, TQ], F32)
    xkv_d = din("xkv", [128, FT, KV], BF16)
    enc_d = din("enc", [128, FT, KV], BF16)
    w_d = {}
    for nm in ("wq_s", "wk_s", "wv_s", "wo_s", "wq_c", "wk_c", "wv_c", "wo_c"):
        w_d[nm] = din(nm, [128, FT, D], BF16)
    w1_d = din("w1", [128, FT, DFF], BF16)
    w2_d = din("w2", [128, HT, D], BF16)
    bias_d = {}
    for nm in ("bq_s", "bk_s", "bv_s", "bo_s", "bq_c", "bk_c", "bv_c", "bo_c",
               "b2", "g1", "be1", "g2", "be2", "g3", "be3"):
        bias_d[nm] = din(nm, [128, FT], F32)
    b1_d = din("b1", [128, HT], F32)
    biasa_d = din("biasa", [128, 16], F32)
    biasb_d = din("biasb", [128, 16], F32)
    dmask_d = din("dmask", [128, 512], BF16)
    out_d = nc.dram_tensor("out", [128, FT, TQ], F32, kind="ExternalOutput").ap()

    with tile.TileContext(nc) as tc, ExitStack() as S:
        const = S.enter_context(tc.tile_pool(name="const", bufs=1))
        pp = S.enter_context(tc.tile_pool(name="ps", bufs=1, space="PSUM"))
        resid = S.enter_context(tc.tile_pool(name="resid", bufs=1))

        # ---- constants ----
        ident = const.tile([128, 128], BF16)
        make_identity(nc, ident)
        ones_col = const.tile([128, 1], F32)
        nc.vector.memset(ones_col, 1.0)
        ones_row = const.tile([1, 128], F32)
        nc.vector.memset(ones_row, 1.0)
        eps_t = const.tile([1, 1], F32)
        nc.vector.memset(eps_t, 1e-5)
        b_sb = {}
        for nm in bias_d:
            b_sb[nm] = const.tile([128, FT], F32, name=f"c_{nm}")
            nc.sync.dma_start(out=b_sb[nm], in_=bias_d[nm])
        b1_sb = const.tile([128, HT], F32)
        nc.sync.dma_start(out=b1_sb, in_=b1_d)
        biasa_sb = const.tile([128, 16], F32)
        nc.sync.dma_start(out=biasa_sb, in_=biasa_d)
        biasb_sb = const.tile([128, 16], F32)
        nc.sync.dma_start(out=biasb_sb, in_=biasb_d)
        dmask_sb = const.tile([128, 512], BF16)
        nc.sync.dma_start(out=dmask_sb, in_=dmask_d)

        # ---- residual-stream tiles ----
        x_res = resid.tile([128, FT, TQ], F32, tag="res", bufs=2)
        nc.sync.dma_start(out=x_res, in_=xres_d)

        # =========== helpers ===========
        def ps_tile(tag, bufs, shape=(128, 512), dt=F32, name="ps"):
            return pp.tile(list(shape), dt, tag=tag, bufs=bufs, name=name)

        def proj_f(out_t, W_sb, X_sb, bias_t, free, copy_eng):
            """out_t[:, ft, :] = W.T @ X + bias  (feature-major, free<=2048)"""
            nseg = (free + 511) // 512
            for ft in range(FT):
                for seg in range(nseg):
                    w = min(512, free - seg * 512)
                    sl = slice(seg * 512, seg * 512 + w)
                    ps = ps_tile("big", 2, name=f"pj_{ft}_{seg}")
                    for dc in range(FT):
                        nc.tensor.matmul(
                            ps[:, :w],
                            lhsT=W_sb[:, dc, ft * 128:(ft + 1) * 128],
                            rhs=X_sb[:, dc, sl],
                            start=(dc == 0), stop=(dc == FT - 1))
                    if copy_eng == "act":
                        nc.scalar.activation(out=out_t[:, ft, sl], in_=ps[:, :w],
                                             func=AF.Identity,
                                             bias=bias_t[:, ft:ft + 1], scale=1.0)
                    else:
                        nc.vector.tensor_scalar_add(out=out_t[:, ft, sl],
                                                    in0=ps[:, :w],
                                                    scalar1=bias_t[:, ft:ft + 1])

        def proj_v(V_list, X_sb, WV_sb, vpool, vtag, vbufs):
            """token-major V (+ones col): V_list[st] = [128s, H, DK+1]"""
            for st in range(KV // 128):
                vt = vpool.tile([128, H, DK + 1], BF16, tag=vtag, bufs=vbufs,
                                name=f"v_{vtag}_{st}")
                for half in range(2):
                    ps = ps_tile("big" if half == 0 else "st", 2,
                                 name=f"pv{st}_{half}")
                    for dc in range(FT):
                        nc.tensor.matmul(
                            ps,
                            lhsT=X_sb[:, dc, st * 128:(st + 1) * 128],
                            rhs=WV_sb[:, dc, half * 512:(half + 1) * 512],
                            start=(dc == 0), stop=(dc == FT - 1))
                    nc.vector.tensor_copy(
                        out=vt[:, half * 8:(half + 1) * 8, 0:DK],
                        in_=ps.rearrange("p (a b) -> p a b", b=DK))
                nc.vector.memset(vt[:, :, DK:DK + 1], 1.0)
                V_list.append(vt)

        def attention(QT, KT, V_list, attnT, apool, atag, abufs, awidth,
                      chunks, bv_t, label):
            """chunks: list of (qoff, qlen, s_tiles, bias_sb|None, diag{st:off})"""
            for h in range(H):
                fp, po = h // 2, (h % 2) * DK
                for ci, (qoff, qlen, s_tiles, bias_sb, diag) in enumerate(chunks):
                    ats = {}
                    for st in s_tiles:
                        ps = ps_tile("st", 2, name=f"ps_{label}_{h}_{ci}_{st}")
                        nc.tensor.matmul(
                            ps[:, :qlen],
                            lhsT=KT[po:po + DK, fp, st * 128:(st + 1) * 128],
                            rhs=QT[po:po + DK, fp, qoff:qoff + qlen],
                            start=True, stop=True)
                        at = apool.tile([128, awidth], BF16, tag=atag,
                                        bufs=abufs, name=f"a_{label}_{h}_{ci}_{st}")
                        if bias_sb is not None:
                            nc.scalar.activation(out=at[:, :qlen], in_=ps[:, :qlen],
                                                 func=AF.Exp, scale=0.125,
                                                 bias=bias_sb[:, st:st + 1])
                        else:
                            nc.scalar.activation(out=at[:, :qlen], in_=ps[:, :qlen],
                                                 func=AF.Exp, scale=0.125)
                        if st in diag:
                            off = diag[st]
                            nc.vector.tensor_mul(
                                at[:, :qlen], at[:, :qlen],
                                dmask_sb[:, off:off + qlen])
                        ats[st] = at
                    for qt in range(qlen // 128):
                        psav = ps_tile("av", 2, shape=(128, DK + 1),
                                       name=f"pav_{label}_{h}_{ci}_{qt}")
                        for i, st in enumerate(s_tiles):
                            nc.tensor.matmul(
                                psav,
                                lhsT=ats[st][:, qt * 128:(qt + 1) * 128],
                                rhs=V_list[st][:, h, :],
                                start=(i == 0), stop=(i == len(s_tiles) - 1))
                        rec = apool.tile([128, 1], F32, tag="rec", bufs=3,
                                         name=f"rec_{label}_{h}_{ci}_{qt}")
                        nc.vector.reciprocal(rec, psav[:, DK:DK + 1])
                        an = apool.tile([128, DK], BF16, tag="an", bufs=3,
                                        name=f"an_{label}_{h}_{ci}_{qt}")
                        nc.vector.tensor_scalar_mul(an, psav[:, 0:DK], rec)
                        pst = ps_tile("t", 1, shape=(DK, 128), dt=BF16,
                                      name=f"pt_{label}_{h}_{ci}_{qt}")
                        nc.tensor.transpose(pst, an, ident)
                        q0 = qoff + qt * 128
                        nc.scalar.activation(
                            out=attnT[po:po + DK, fp, q0:q0 + 128], in_=pst,
                            func=AF.Identity, bias=bv_t[po:po + DK, fp:fp + 1],
                            scale=1.0)

        def wo_resid(attnT, WO_sb, bo_t, x_prev, x_out):
            """x_out = x_prev + attnT.T@Wo + bo   (all f32)"""
            for fo in range(FT):
                ps = ps_tile("big", 2, name=f"pwo_{fo}")
                for fi in range(FT):
                    nc.tensor.matmul(ps,
                                     lhsT=WO_sb[:, fi, fo * 128:(fo + 1) * 128],
                                     rhs=attnT[:, fi, :],
                                     start=(fi == 0), stop=(fi == FT - 1))
                nc.vector.scalar_tensor_tensor(
                    out=x_out[:, fo, :], in0=ps, scalar=bo_t[:, fo:fo + 1],
                    in1=x_prev[:, fo, :], op0=ALU.add, op1=ALU.add)

        def layernorm(x_in, out_t, g_t, be_t, out_dt_label):
            """out_t[:, fc, :] = LN(x_in) * g + be  (stats via f32r matmuls)"""
            ps_sum = ps_tile("st", 2, shape=(1, TQ), name=f"psum_{out_dt_label}")
            for fc in range(FT):
                nc.tensor.matmul(ps_sum, lhsT=ones_col,
                                 rhs=x_in[:, fc, :],
                                 start=(fc == 0), stop=(fc == FT - 1))
            mu = resid.tile([1, TQ], F32, tag="stat", bufs=2,
                            name=f"mu_{out_dt_label}")
            nc.scalar.activation(out=mu, in_=ps_sum, func=AF.Copy, scale=1.0 / D)
            ps_sq = ps_tile("st", 2, shape=(1, TQ), name=f"psq_{out_dt_label}")
            for fc in range(FT):
                sq = resid.tile([128, TQ], F32, tag="sq", bufs=2,
                                name=f"sq_{out_dt_label}_{fc}")
                nc.vector.tensor_mul(sq, x_in[:, fc, :], x_in[:, fc, :])
                nc.tensor.matmul(ps_sq, lhsT=ones_col,
                                 rhs=sq,
                                 start=(fc == 0), stop=(fc == FT - 1))
            msq = resid.tile([1, TQ], F32, tag="stat", bufs=2,
                             name=f"msq_{out_dt_label}")
            nc.scalar.activation(out=msq, in_=ps_sq, func=AF.Copy, scale=1.0 / D)
            mu2 = resid.tile([128, TQ], F32, tag="sq", bufs=2,
                             name=f"mu2_{out_dt_label}")
            nc.vector.tensor_mul(mu2[0:1, :], mu, mu)
            nc.vector.tensor_sub(msq, msq, mu2[0:1, :])  # msq <- var
            nc.scalar.activation(out=msq, in_=msq, func=AF.Sqrt, bias=eps_t,
                                 scale=1.0)              # msq <- std
            ps_mu = ps_tile("misc", 1, name=f"pmu_{out_dt_label}")
            nc.tensor.matmul(ps_mu, lhsT=ones_row,
                             rhs=mu, start=True, stop=True)
            rstd = resid.tile([1, TQ], F32, tag="stat", bufs=2,
                              name=f"rstd_{out_dt_label}")
            nc.vector.reciprocal(rstd, msq)
            ps_rstd = ps_tile("t", 1, name=f"prstd_{out_dt_label}")
            nc.tensor.matmul(ps_rstd, lhsT=ones_row,
                             rhs=rstd, start=True, stop=True)
            for fc in range(FT):
                tmp = resid.tile([128, TQ], F32, tag="sq", bufs=2,
                                 name=f"t_{out_dt_label}_{fc}")
                nc.vector.tensor_sub(tmp, x_in[:, fc, :], ps_mu)
                nc.vector.tensor_mul(tmp, tmp, ps_rstd)
                nc.vector.tensor_scalar(out=out_t[:, fc, :], in0=tmp,
                                        scalar1=g_t[:, fc:fc + 1],
                                        scalar2=be_t[:, fc:fc + 1],
                                        op0=ALU.mult, op1=ALU.add)

        # =========== program ===========
        glob_ctx = ExitStack()
        glob = glob_ctx.enter_context(tc.tile_pool(name="glob", bufs=1))

        chunks_self = [
            (0, CHUNK, SA_TILES, biasa_sb, DIAG_A),
            (CHUNK, CHUNK, SB_TILES, biasb_sb, DIAG_B),
        ]
        chunks_cross = [(0, TQ, list(range(KV // 128)), None, {})]

        # ---- phase 1: self-attn projections ----
        QT = glob.tile([128, FT, TQ], BF16, tag="qt", bufs=1, name="QT_s")
        KT = glob.tile([128, FT, KV], BF16, tag="kt", bufs=1, name="KT_s")
        V_s = []
        with ExitStack() as S1:
            wp = S1.enter_context(tc.tile_pool(name="wself", bufs=1))
            xq_b = wp.tile([128, FT, TQ], BF16, tag="xq", bufs=1)
            nc.sync.dma_start(out=xq_b, in_=xq_d)
            xkv = wp.tile([128, FT, KV], BF16, tag="xkv", bufs=1)
            nc.sync.dma_start(out=xkv, in_=xkv_d)
            wq = wp.tile([128, FT, D], BF16, tag="wproj", bufs=1, name="wq_s")
            nc.sync.dma_start(out=wq, in_=w_d["wq_s"])
            proj_f(QT, wq, xq_b, b_sb["bq_s"], TQ, "act")
            wk = wp.tile([128, FT, D], BF16, tag="wproj", bufs=1, name="wk_s")
            nc.sync.dma_start(out=wk, in_=w_d["wk_s"])
            proj_f(KT, wk, xkv, b_sb["bk_s"], KV, "dve")
            wv = wp.tile([128, FT, D], BF16, tag="wproj", bufs=1, name="wv_s")
            nc.sync.dma_start(out=wv, in_=w_d["wv_s"])
            proj_v(V_s, xkv, wv, glob, "v", 16)

        # ---- phase 2: self-attention + Wo + LN1 ----
        attnT = glob.tile([128, FT, TQ], BF16, tag="attnT", bufs=1, name="attnT_s")
        x1p = resid.tile([128, FT, TQ], F32, tag="res", bufs=2, name="x1p")
        x1f = resid.tile([128, FT, TQ], F32, tag="res", bufs=2, name="x1f")
        x1n = resid.tile([128, FT, TQ], BF16, tag="xn", bufs=2, name="x1n")
        with ExitStack() as S2:
            ap2 = S2.enter_context(tc.tile_pool(name="aself", bufs=1))
            attention(QT, KT, V_s, attnT, ap2, "at_s", 28, CHUNK,
                      chunks_self, b_sb["bv_s"], "s")
            wo = ap2.tile([128, FT, D], BF16, tag="wo", bufs=1, name="wo_s")
            nc.sync.dma_start(out=wo, in_=w_d["wo_s"])
            wo_resid(attnT, wo, b_sb["bo_s"], x_res, x1p)
            layernorm(x1p, x1f, b_sb["g1"], b_sb["be1"], "ln1")
            for fc in range(FT):
                nc.vector.tensor_copy(out=x1n[:, fc, :], in_=x1f[:, fc, :])

        # ---- phase 3: cross-attn projections ----
        QT_c = glob.tile([128, FT, TQ], BF16, tag="qt", bufs=1, name="QT_c")
        KT_c = glob.tile([128, FT, KV], BF16, tag="kt", bufs=1, name="KT_c")
        V_c = []
        with ExitStack() as S3:
            wp3 = S3.enter_context(tc.tile_pool(name="wcross", bufs=1))
            enc_b = wp3.tile([128, FT, KV], BF16, tag="enc", bufs=1)
            nc.sync.dma_start(out=enc_b, in_=enc_d)
            wqc = wp3.tile([128, FT, D], BF16, tag="wprojc", bufs=1, name="wq_c")
            nc.sync.dma_start(out=wqc, in_=w_d["wq_c"])
            proj_f(QT_c, wqc, x1n, b_sb["bq_c"], TQ, "act")
            wkc = wp3.tile([128, FT, D], BF16, tag="wprojc", bufs=1, name="wk_c")
            nc.sync.dma_start(out=wkc, in_=w_d["wk_c"])
            proj_f(KT_c, wkc, enc_b, b_sb["bk_c"], KV, "dve")
            wvc = wp3.tile([128, FT, D], BF16, tag="wprojc", bufs=1, name="wv_c")
            nc.sync.dma_start(out=wvc, in_=w_d["wv_c"])
            proj_v(V_c, enc_b, wvc, glob, "v", 16)

        # ---- phase 4: cross-attention + Wo + LN2 ----
        attnT_c = glob.tile([128, FT, TQ], BF16, tag="attnT", bufs=1,
                            name="attnT_c")
        x2p = resid.tile([128, FT, TQ], F32, tag="res", bufs=2, name="x2p")
        x2f = resid.tile([128, FT, TQ], F32, tag="res", bufs=2, name="x2f")
        x2n = resid.tile([128, FT, TQ], BF16, tag="xn", bufs=2, name="x2n")
        with ExitStack() as S4:
            ap4 = S4.enter_context(tc.tile_pool(name="across", bufs=1))
            attention(QT_c, KT_c, V_c, attnT_c, ap4, "at_c", 20, TQ,
                      chunks_cross, b_sb["bv_c"], "c")
            woc = ap4.tile([128, FT, D], BF16, tag="woc", bufs=1, name="wo_c")
            nc.sync.dma_start(out=woc, in_=w_d["wo_c"])
            wo_resid(attnT_c, woc, b_sb["bo_c"], x1f, x2p)
            layernorm(x2p, x2f, b_sb["g2"], b_sb["be2"], "ln2")
            for fc in range(FT):
                nc.vector.tensor_copy(out=x2n[:, fc, :], in_=x2f[:, fc, :])

        glob_ctx.close()

        # ---- phase 5: FFN + LN3 + output ----
        x3 = resid.tile([128, FT, TQ], F32, tag="res", bufs=2, name="x3")
        out_sb = resid.tile([128, FT, TQ], F32, tag="res", bufs=2, name="out_sb")
        with ExitStack() as S5:
            fp5 = S5.enter_context(tc.tile_pool(name="ffn", bufs=1))
            h_sb = fp5.tile([128, HT, TQ], BF16, tag="h", bufs=1, name="h_sb")
            w2 = fp5.tile([128, HT, D], BF16, tag="w2", bufs=1, name="w2")
            nc.sync.dma_start(out=w2, in_=w2_d)
            for g in range(4):  # stream W1 in 4 pieces of [128, FT, 1024]
                w1p = fp5.tile([128, FT, 1024], BF16, tag="w1", bufs=2,
                               name=f"w1_{g}")
                nc.sync.dma_start(
                    out=w1p, in_=w1_d[:, :, g * 1024:(g + 1) * 1024])
                for i in range(8):  # 8 ffn f-tiles per piece
                    ht = g * 8 + i
                    ps = ps_tile("big", 2, name=f"pf1_{ht}")
                    for dc in range(FT):
                        nc.tensor.matmul(
                            ps, lhsT=w1p[:, dc, i * 128:(i + 1) * 128],
                            rhs=x2n[:, dc, :],
                            start=(dc == 0), stop=(dc == FT - 1))
                    # bias-add + relu + bf16 cast in one DVE op
                    nc.vector.tensor_scalar(out=h_sb[:, ht, :], in0=ps,
                                            scalar1=b1_sb[:, ht:ht + 1],
                                            scalar2=0.0,
                                            op0=ALU.add, op1=ALU.max)
            for fo in range(FT):
                ps = ps_tile("big", 2, name=f"pf2_{fo}")
                for ht in range(HT):
                    nc.tensor.matmul(ps, lhsT=w2[:, ht, fo * 128:(fo + 1) * 128],
                                     rhs=h_sb[:, ht, :],
                                     start=(ht == 0), stop=(ht == HT - 1))
                nc.vector.scalar_tensor_tensor(
                    out=x3[:, fo, :], in0=ps, scalar=b_sb["b2"][:, fo:fo + 1],
                    in1=x2f[:, fo, :], op0=ALU.add, op1=ALU.add)
            layernorm(x3, out_sb, b_sb["g3"], b_sb["be3"], "ln3")
            nc.sync.dma_start(out=out_d, in_=out_sb)

    nc.compile()
    return nc


def _to_tiles(a2d, dt=ml_dtypes.bfloat16):
    """[P*128, F] -> [128, P, F] (SBUF tile layout), casting to dt."""
    p8, f = a2d.shape
    return np.ascontiguousarray(
        a2d.reshape(p8 // 128, 128, f).transpose(1, 0, 2).astype(dt))


def _vec_tiles(v, dt=np.float32):
    """[n*128] -> [128, n]"""
    return np.ascontiguousarray(v.reshape(-1, 128).T.astype(dt))


def _prep_core(c, dec, enc, consts):
    j = c % 4
    b = c // 4
    ja, jb = j, 7 - j
    rest = [ch for ch in range(0, jb) if ch != ja]
    qtok = np.r_[ja * CHUNK:(ja + 1) * CHUNK, jb * CHUNK:(jb + 1) * CHUNK]
    kvtok = np.concatenate(
        [qtok] + [np.arange(ch * CHUNK, (ch + 1) * CHUNK) for ch in rest])
    xq = dec[b][qtok]                       # [512, D]
    xkv = np.zeros((KV, D), np.float32)
    xkv[: len(kvtok)] = dec[b][kvtok]
    real_len = len(kvtok)

    biasa = np.zeros(KV, np.float32)
    biasa[512 + ja * CHUNK:] = NEG
    biasb = np.zeros(KV, np.float32)
    biasb[real_len:] = NEG

    m = dict(consts)
    m["xq"] = _to_tiles(xq.T)
    m["xres"] = _to_tiles(xq.T, np.float32)
    m["xkv"] = _to_tiles(xkv.T)
    m["enc"] = _to_tiles(enc[b].T)
    m["biasa"] = _vec_tiles(biasa)
    m["biasb"] = _vec_tiles(biasb)
    return m, (b, qtok)


def _prep_consts(inputs):
    bf = ml_dtypes.bfloat16
    c = {}
    for src, dst in (("Wq_s", "wq_s"), ("Wk_s", "wk_s"), ("Wv_s", "wv_s"),
                     ("Wq_c", "wq_c"), ("Wk_c", "wk_c"), ("Wv_c", "wv_c")):
        w = np.asarray(inputs[src], np.float32)           # [H, D, DK]
        c[dst] = _to_tiles(w.transpose(1, 0, 2).reshape(D, D))
    c["wo_s"] = _to_tiles(np.asarray(inputs["Wo_s"], np.float32))
    c["wo_c"] = _to_tiles(np.asarray(inputs["Wo_c"], np.float32))
    c["w1"] = _to_tiles(np.asarray(inputs["W1"], np.float32))
    c["w2"] = _to_tiles(np.asarray(inputs["W2"], np.float32))
    for src, dst in (("bq_s", "bq_s"), ("bk_s", "bk_s"), ("bv_s", "bv_s"),
                     ("bo_s", "bo_s"), ("bq_c", "bq_c"), ("bk_c", "bk_c"),
                     ("bv_c", "bv_c"), ("bo_c", "bo_c"), ("b2", "b2"),
                     ("g1", "g1"), ("be1", "be1"), ("g2", "g2"),
                     ("be2", "be2"), ("g3", "g3"), ("be3", "be3")):
        c[dst] = _vec_tiles(np.asarray(inputs[src], np.float32).reshape(-1))
    c["b1"] = _vec_tiles(np.asarray(inputs["b1"], np.float32))
    # causal diag mask M[s, q] = 1 if s <= q, packed [128, 512]
    M = (np.arange(CHUNK)[:, None] <= np.arange(CHUNK)[None, :]).astype(bf)
    c["dmask"] = np.ascontiguousarray(
        np.concatenate([M[0:128], M[128:256]], axis=1))
    return c


def kernel(**inputs):
    global _BUILT
    if _BUILT is None:
        _BUILT = _build()
    nc = _BUILT

    dec = np.asarray(inputs["dec_input"], np.float32)
    enc = np.asarray(inputs["enc_output"], np.float32)
    consts = _prep_consts(inputs)
    in_maps = []
    metas = []
    for cix in range(N_CORES):
        m, meta = _prep_core(cix, dec, enc, consts)
        in_maps.append(m)
        metas.append(meta)

    res = run_bass_kernel_spmd(nc, in_maps, core_ids=list(range(N_CORES)))

    out = np.empty((B, T, D), np.float32)
    for cix in range(N_CORES):
        b, qtok = metas[cix]
        tiles = res.results[cix]["out"]       # [128, FT, TQ]
        core_t = tiles.transpose(1, 0, 2).reshape(D, TQ)
        out[b, qtok, :] = core_t.T
    return out


# revision 42
# speedup vs baseline: 1.1084x; 1.1084x over previous
"""Transformer decoder layer (self-attn + cross-attn + FFN, post-LN) on 8
Trainium2 NeuronCores, sequence-parallel with zero collectives.

Sharding: core c -> batch b = c//4, causal-balanced chunk pair (j, 7-j) of
256 tokens each (j = c%4), so every core owns 512 query tokens with equal
total causal attention area. Weights are replicated; K/V projections are
recomputed per core. All per-core differences are expressed through input
DATA (token reordering + additive exp-bias masks), so a single SPMD program
serves all 8 cores.

Layout: activations are kept feature-major [d_partition, token_free] so
every matmul contracts along SBUF partitions with weights as the stationary
operand. Scores are computed transposed (S^T = [s, q]) which makes softmax
need no cross-partition reduction: exp on ScalarE (scores are O(0.5), so no
max subtraction), denominator via an extra ones-column appended to V.
Matmuls run in bf16 with fp32 PSUM accumulation; the residual stream and
layernorm arithmetic stay fp32 (stat sums in bf16, mean/rstd broadcasts via
fp32 matmuls).

Overlap structure: chunk-A attention (which only needs the first 1280 kv
tokens) is emitted mid-way through the K/V projection segments; the whole
cross K/V projection is sandwiched between LN1's stats and apply so the LN
latency chain hides under projection matmuls.
"""

import sys

if "/opt/trn_rl_repo" not in sys.path:
    sys.path.insert(0, "/opt/trn_rl_repo")

from contextlib import ExitStack

import numpy as np
import ml_dtypes

import concourse.bass as bass
import concourse.bacc as bacc
import concourse.tile as tile
import concourse.mybir as mybir
from concourse.bass_utils import run_bass_kernel_spmd
from concourse.masks import make_identity

F32 = mybir.dt.float32
BF16 = mybir.dt.bfloat16
AF = mybir.ActivationFunctionType
ALU = mybir.AluOpType

D = 1024
H = 16
DK = 64
DFF = 4096
B = 2
T = 2048
N_CORES = 8
CHUNK = 256
TQ = 512          # query tokens per core
KV = 2048         # padded kv layout length (self), enc length (cross)
FT = D // 128     # 8 f-tiles
HT = DFF // 128   # 32 ffn tiles
NSEG = 8          # kv/enc DMA-streaming segments of 256 tokens
NEG = -50.0       # additive pre-exp mask (exp(-50) ~ 2e-22)

# self-attn 256-token s-block schedules over the kv layout
# [A(256) | B(256) | rest... | pad]  (block = 2 s-tiles of 128):
BLOCKS_A = [0, 2, 3, 4]    # own diag + 768-token prior window
BLOCKS_B = list(range(8))  # everything (pads masked via bias)

_BUILT = None


def _build():
    nc = bacc.Bacc("TRN2", target_bir_lowering=False, debug=False,
                   num_devices=N_CORES)

    def din(name, shape, dt):
        return nc.dram_tensor(name, shape, dt, kind="ExternalInput").ap()

    xq_d = din("xq", [128, FT, TQ], BF16)
    xres_d = din("xres", [128, FT, TQ], F32)
    xkv_d = din("xkv", [NSEG, 128, FT, 256], BF16)    # seg-major
    enc_d = din("enc", [NSEG, 128, FT, 256], BF16)    # seg-major
    w_d = {}
    for nm in ("wq_s", "wk_s", "wv_s", "wo_s", "wq_c", "wk_c", "wv_c", "wo_c"):
        w_d[nm] = din(nm, [128, FT, D], BF16)
    w1_d = din("w1", [128, FT, DFF], BF16)
    w2_d = din("w2", [128, HT, D], BF16)
    # all small fp32 vectors packed into one tensor: 15 biases/ln params of
    # [128, 8], then b1 [128, 32], biasa2 [128, 8], biasb2 [128, 8]
    BIAS_NAMES = ("bq_s", "bk_s", "bv_s", "bo_s", "bq_c", "bk_c", "bv_c",
                  "bo_c", "b2", "g1", "be1", "g2", "be2", "g3", "be3")
    smallf_d = din("smallf", [128, 15 * FT + HT + 16], F32)
    dmask_d = din("dmask", [128, 512], BF16)
    out_d = nc.dram_tensor("out", [128, FT, TQ], F32, kind="ExternalOutput").ap()

    with tile.TileContext(nc) as tc, ExitStack() as S:
        const = S.enter_context(tc.tile_pool(name="const", bufs=1))
        pp = S.enter_context(tc.tile_pool(name="ps", bufs=1, space="PSUM"))
        resid = S.enter_context(tc.tile_pool(name="resid", bufs=1))

        ident = const.tile([128, 128], BF16)
        make_identity(nc, ident)
        ones_b = const.tile([128, 1], BF16)
        nc.vector.memset(ones_b, 1.0)
        ones_row = const.tile([1, 128], F32)
        nc.vector.memset(ones_row, 1.0)
        eps_t = const.tile([1, 1], F32)
        nc.vector.memset(eps_t, 1e-5)

        glob_ctx = ExitStack()
        glob = glob_ctx.enter_context(tc.tile_pool(name="glob", bufs=1))

        # =========== helpers ===========
        def ps_tile(tag, bufs, shape=(128, 512), dt=F32, name="ps"):
            return pp.tile(list(shape), dt, tag=tag, bufs=bufs, name=name)

        def wtile(nm):
            t = glob.tile([128, FT, D], BF16, tag="wstream", bufs=2, name=nm)
            for dc in range(FT):     # per-chunk so first consumers start early
                nc.sync.dma_start(out=t[:, dc, :], in_=w_d[nm][:, dc, :])
            return t

        TAG8 = ["big", "big", "st", "st", "av", "av", "t", "misc"]
        BUF8 = [2, 2, 2, 2, 2, 2, 1, 1]

        def proj_q(out_t, W_sb, X_sb, bias_t, lbl):
            # dc-outer with 8 concurrent accumulators: the first matmul only
            # needs the first d-chunk of W and X (fast start after DMA).
            ps8 = [ps_tile(TAG8[ft], BUF8[ft], name=f"pjq_{lbl}_{ft}")
                   for ft in range(FT)]
            for dc in range(FT):
                for ft in range(FT):
                    nc.tensor.matmul(
                        ps8[ft], lhsT=W_sb[:, dc, ft * 128:(ft + 1) * 128],
                        rhs=X_sb[:, dc, :],
                        start=(dc == 0), stop=(dc == FT - 1))
            for ft in range(FT):
                nc.scalar.activation(out=out_t[:, ft, :], in_=ps8[ft],
                                     func=AF.Identity,
                                     bias=bias_t[:, ft:ft + 1], scale=1.0)

        def proj_kv_seg(KT, V_list, seg, X_piece, WK_sb, WV_sb, bk_t, vtag):
            """one 256-token segment of V (token-major) then K^T (f-major)."""
            sl = slice(seg * 256, (seg + 1) * 256)
            for sti in range(2):
                st = seg * 2 + sti
                vt = glob.tile([128, H, DK + 1], BF16, tag="v", bufs=16,
                               name=f"v_{vtag}_{st}")
                for half in range(2):
                    ps = ps_tile("big" if half == 0 else "st", 2,
                                 name=f"pv_{vtag}_{st}_{half}")
                    for dc in range(FT):
                        nc.tensor.matmul(
                            ps,
                            lhsT=X_piece[:, dc, sti * 128:(sti + 1) * 128],
                            rhs=WV_sb[:, dc, half * 512:(half + 1) * 512],
                            start=(dc == 0), stop=(dc == FT - 1))
                    nc.vector.tensor_copy(
                        out=vt[:, half * 8:(half + 1) * 8, 0:DK],
                        in_=ps.rearrange("p (a b) -> p a b", b=DK))
                nc.vector.memset(vt[:, :, DK:DK + 1], 1.0)
                V_list.append(vt)
            for ft in range(FT):
                ps = ps_tile("big" if ft % 2 == 0 else "st", 2,
                             shape=(128, 256), name=f"pjk_{vtag}_{seg}_{ft}")
                for dc in range(FT):
                    nc.tensor.matmul(
                        ps, lhsT=WK_sb[:, dc, ft * 128:(ft + 1) * 128],
                        rhs=X_piece[:, dc, :],
                        start=(dc == 0), stop=(dc == FT - 1))
                nc.vector.tensor_scalar_add(out=KT[:, ft, sl], in0=ps,
                                            scalar1=bk_t[:, ft:ft + 1])

        # Normalized attention tiles go through a PE transpose whose input
        # comes from a short DVE chain; emitting the transpose immediately
        # would stall the in-order PE stream on DVE. Instead stage-1 (DVE
        # recip+scale) is emitted with the AV matmuls and the transposes are
        # deferred into the NEXT head's PE stream.
        pending_t = []
        _tcnt = [0]

        def _norm1(psav, attnT, po, fp, q0, bv_t, nm):
            rec = glob.tile([128, 1], F32, tag="rec", bufs=10, name=f"r{nm}")
            nc.vector.reciprocal(rec, psav[:, DK:DK + 1])
            an = glob.tile([128, DK], BF16, tag="an", bufs=10, name=f"n{nm}")
            nc.vector.tensor_scalar_mul(an, psav[:, 0:DK], rec)
            pending_t.append((an, attnT, po, fp, q0, bv_t))

        def flush_t():
            for an, attnT, po, fp, q0, bv_t in pending_t:
                _tcnt[0] += 1
                pst = ps_tile("t" if _tcnt[0] % 2 == 0 else "misc", 1,
                              shape=(DK, 128), dt=BF16, name=f"pt{_tcnt[0]}")
                nc.tensor.transpose(pst, an, ident)
                nc.vector.tensor_scalar_add(
                    out=attnT[po:po + DK, fp, q0:q0 + 128], in0=pst,
                    scalar1=bv_t[po:po + DK, fp:fp + 1])
            pending_t.clear()

        def attn_chunk(QT, KT, V_list, attnT, bv_t, cn, qoff, blocks, bias2,
                       diag_blk):
            for h in range(H):
                fp, po = h // 2, (h % 2) * DK
                ats = {}
                for blk in blocks:
                    ps = ps_tile("st", 2, name=f"pss_{h}_{cn}_{blk}")
                    for half in range(2):
                        st = blk * 2 + half
                        nc.tensor.matmul(
                            ps[:, half * 256:(half + 1) * 256],
                            lhsT=KT[po:po + DK, fp, st * 128:(st + 1) * 128],
                            rhs=QT[po:po + DK, fp, qoff:qoff + CHUNK],
                            start=True, stop=True)
                    at = glob.tile([128, 512], BF16, tag="at", bufs=16,
                                   name=f"a_{h}_{cn}_{blk}")
                    nc.scalar.activation(out=at, in_=ps, func=AF.Exp,
                                         scale=0.125,
                                         bias=bias2[:, blk:blk + 1])
                    if blk == diag_blk:
                        nc.vector.tensor_mul(at, at, dmask_sb)
                    ats[blk] = at
                flush_t()
                for qt in range(2):
                    psav = ps_tile("av" if qt % 2 == 0 else "big", 2,
                                   shape=(128, DK + 1),
                                   name=f"pav_{h}_{cn}_{qt}")
                    units = [(blk, half) for blk in blocks
                             for half in range(2)]
                    for i, (blk, half) in enumerate(units):
                        st = blk * 2 + half
                        nc.tensor.matmul(
                            psav,
                            lhsT=ats[blk][:, half * 256 + qt * 128:
                                          half * 256 + (qt + 1) * 128],
                            rhs=V_list[st][:, h, :],
                            start=(i == 0), stop=(i == len(units) - 1))
                    _norm1(psav, attnT, po, fp, qoff + qt * 128, bv_t,
                           f"s_{h}_{cn}_{qt}")
            flush_t()

        def attn_cross(QT, KT, V_list, attnT, bv_t):
            # two half-passes over s so only 8 exp tiles are live per head
            for h in range(H):
                fp, po = h // 2, (h % 2) * DK
                psavs = [ps_tile("av" if qt % 2 == 0 else "big", 2,
                                 shape=(128, DK + 1), name=f"pavc_{h}_{qt}")
                         for qt in range(4)]
                for half in range(2):
                    ats = {}
                    for st in range(half * 8, half * 8 + 8):
                        ps = ps_tile("st", 2, name=f"psc_{h}_{st}")
                        nc.tensor.matmul(
                            ps,
                            lhsT=KT[po:po + DK, fp, st * 128:(st + 1) * 128],
                            rhs=QT[po:po + DK, fp, :], start=True, stop=True)
                        at = glob.tile([128, 512], BF16, tag="at", bufs=16,
                                       name=f"ac_{h}_{st}")
                        nc.scalar.activation(out=at, in_=ps, func=AF.Exp,
                                             scale=0.125)
                        ats[st] = at
                    if half == 0:
                        flush_t()
                    for qt in range(4):
                        for st in range(half * 8, half * 8 + 8):
                            nc.tensor.matmul(
                                psavs[qt],
                                lhsT=ats[st][:, qt * 128:(qt + 1) * 128],
                                rhs=V_list[st][:, h, :],
                                start=(st == 0), stop=(st == 15))
                for qt in range(4):
                    _norm1(psavs[qt], attnT, po, fp, qt * 128, bv_t,
                           f"c_{h}_{qt}")
            flush_t()

        def wo_resid(attnT, WO_sb, bo_t, x_prev, x_out):
            for fo in range(FT):
                ps = ps_tile("big" if fo % 2 == 0 else "st", 2,
                             name=f"pwo_{fo}")
                for fi in range(FT):
                    nc.tensor.matmul(ps,
                                     lhsT=WO_sb[:, fi, fo * 128:(fo + 1) * 128],
                                     rhs=attnT[:, fi, :],
                                     start=(fi == 0), stop=(fi == FT - 1))
                nc.vector.scalar_tensor_tensor(
                    out=x_out[:, fo, :], in0=ps, scalar=bo_t[:, fo:fo + 1],
                    in1=x_prev[:, fo, :], op0=ALU.add, op1=ALU.add)

        def ln_stats(x_in, lbl):
            """-> (ps_mu, ps_rstd) broadcast PSUM tiles (tags misc/t)."""
            ps_sum = ps_tile("st", 2, shape=(1, TQ), name=f"psum_{lbl}")
            ps_sq = ps_tile("big", 2, shape=(1, TQ), name=f"psq_{lbl}")
            for fc in range(FT):
                xb = resid.tile([128, TQ], BF16, tag="sqb", bufs=3,
                                name=f"xb_{lbl}_{fc}")
                nc.vector.tensor_copy(out=xb, in_=x_in[:, fc, :])
                nc.tensor.matmul(ps_sum, lhsT=ones_b, rhs=xb,
                                 start=(fc == 0), stop=(fc == FT - 1))
                sqb = resid.tile([128, TQ], BF16, tag="sqb", bufs=3,
                                 name=f"sq_{lbl}_{fc}")
                nc.vector.tensor_mul(sqb, xb, xb)
                nc.tensor.matmul(ps_sq, lhsT=ones_b, rhs=sqb,
                                 start=(fc == 0), stop=(fc == FT - 1))
            mu = resid.tile([1, TQ], F32, tag="stat", bufs=2, name=f"mu_{lbl}")
            nc.scalar.activation(out=mu, in_=ps_sum, func=AF.Copy, scale=1.0 / D)
            msq = resid.tile([1, TQ], F32, tag="stat", bufs=2,
                             name=f"msq_{lbl}")
            nc.scalar.activation(out=msq, in_=ps_sq, func=AF.Copy, scale=1.0 / D)
            mu2 = resid.tile([128, TQ], F32, tag="sq", bufs=2,
                             name=f"mu2_{lbl}")
            nc.vector.tensor_mul(mu2[0:1, :], mu, mu)
            nc.vector.tensor_sub(msq, msq, mu2[0:1, :])  # msq <- var
            nc.scalar.activation(out=msq, in_=msq, func=AF.Sqrt, bias=eps_t,
                                 scale=1.0)              # msq <- std
            ps_mu = ps_tile("misc", 1, name=f"pmu_{lbl}")
            nc.tensor.matmul(ps_mu, lhsT=ones_row, rhs=mu, start=True,
                             stop=True)
            rstd = resid.tile([1, TQ], F32, tag="stat", bufs=2,
                              name=f"rstd_{lbl}")
            nc.vector.reciprocal(rstd, msq)
            ps_rstd = ps_tile("t", 1, name=f"prstd_{lbl}")
            nc.tensor.matmul(ps_rstd, lhsT=ones_row, rhs=rstd, start=True,
                             stop=True)
            return ps_mu, ps_rstd

        def ln_apply(stats, x_in, out_t, g_t, be_t, lbl, dma_out=None,
                     bf16_out=None):
            ps_mu, ps_rstd = stats
            for fc in range(FT):
                tmp = resid.tile([128, TQ], F32, tag="sq", bufs=2,
                                 name=f"t_{lbl}_{fc}")
                nc.vector.tensor_sub(tmp, x_in[:, fc, :], ps_mu)
                nc.vector.tensor_mul(tmp, tmp, ps_rstd)
                nc.vector.tensor_scalar(out=out_t[:, fc, :], in0=tmp,
                                        scalar1=g_t[:, fc:fc + 1],
                                        scalar2=be_t[:, fc:fc + 1],
                                        op0=ALU.mult, op1=ALU.add)
                if bf16_out is not None:
                    nc.vector.tensor_copy(out=bf16_out[:, fc, :],
                                          in_=out_t[:, fc, :])
                if dma_out is not None:
                    nc.sync.dma_start(out=dma_out[:, fc, :],
                                      in_=out_t[:, fc, :])

        # =========== program ===========
        QT = glob.tile([128, FT, TQ], BF16, tag="qt", bufs=1, name="QT_s")
        KT = glob.tile([128, FT, KV], BF16, tag="kt", bufs=1, name="KT_s")
        attnT = glob.tile([128, FT, TQ], BF16, tag="attnT", bufs=1,
                          name="attnT_s")
        V_s = []
        x_res = resid.tile([128, FT, TQ], F32, tag="res", bufs=2)
        x1p = resid.tile([128, FT, TQ], F32, tag="res", bufs=2, name="x1p")
        with ExitStack() as S1:
            wp = S1.enter_context(tc.tile_pool(name="wself", bufs=1))
            xq_b = wp.tile([128, FT, TQ], BF16, tag="xq", bufs=1)
            wq = glob.tile([128, FT, D], BF16, tag="wstream", bufs=2,
                           name="wq_s")
            for dc in range(FT):   # per-chunk loads so compute starts early
                nc.sync.dma_start(out=xq_b[:, dc, :], in_=xq_d[:, dc, :])
                nc.sync.dma_start(out=wq[:, dc, :], in_=w_d["wq_s"][:, dc, :])
            # small consts: one packed DMA on the gpsimd queue
            smallf = const.tile([128, 15 * FT + HT + 16], F32, name="c_small")
            nc.gpsimd.dma_start(out=smallf, in_=smallf_d)
            dmask_sb = const.tile([128, 512], BF16, name="c_dm")
            nc.gpsimd.dma_start(out=dmask_sb, in_=dmask_d)
            b_sb = {nm: smallf[:, i * FT:(i + 1) * FT]
                    for i, nm in enumerate(BIAS_NAMES)}
            b1_sb = smallf[:, 15 * FT:15 * FT + HT]
            biasa_sb = smallf[:, 15 * FT + HT:15 * FT + HT + 8]
            biasb_sb = smallf[:, 15 * FT + HT + 8:15 * FT + HT + 16]

            xp0 = wp.tile([128, FT, 256], BF16, tag="xkvp", bufs=2,
                          name="xkv_0")
            nc.sync.dma_start(out=xp0, in_=xkv_d[0])
            proj_q(QT, wq, xq_b, b_sb["bq_s"], "s")
            wv = wtile("wv_s")
            wk = wtile("wk_s")
            proj_kv_seg(KT, V_s, 0, xp0, wk, wv, b_sb["bk_s"], "v")
            nc.sync.dma_start(out=x_res, in_=xres_d)
            for seg in range(1, 5):
                xp = wp.tile([128, FT, 256], BF16, tag="xkvp", bufs=2,
                             name=f"xkv_{seg}")
                nc.sync.dma_start(out=xp, in_=xkv_d[seg])
                proj_kv_seg(KT, V_s, seg, xp, wk, wv, b_sb["bk_s"], "v")
            # chunk-A attention only needs kv tiles 0..9 (segs 0..4)
            attn_chunk(QT, KT, V_s, attnT, b_sb["bv_s"], "A", 0, BLOCKS_A,
                       biasa_sb, 0)
            for seg in range(5, NSEG):
                xp = wp.tile([128, FT, 256], BF16, tag="xkvp", bufs=2,
                             name=f"xkv_{seg}")
                nc.sync.dma_start(out=xp, in_=xkv_d[seg])
                proj_kv_seg(KT, V_s, seg, xp, wk, wv, b_sb["bk_s"], "v")

        attn_chunk(QT, KT, V_s, attnT, b_sb["bv_s"], "B", CHUNK, BLOCKS_B,
                   biasb_sb, 1)
        wo = wtile("wo_s")
        wo_resid(attnT, wo, b_sb["bo_s"], x_res, x1p)

        # LN1 stats now; the whole cross K/V projection runs while the
        # mean/rstd chain resolves; LN1 apply afterwards.
        st1 = ln_stats(x1p, "ln1")
        KT_c = glob.tile([128, FT, KV], BF16, tag="kt", bufs=1, name="KT_c")
        V_c = []
        wvc = wtile("wv_c")
        wkc = wtile("wk_c")
        for seg in range(NSEG):
            ep = glob.tile([128, FT, 256], BF16, tag="encp", bufs=2,
                           name=f"enc_{seg}")
            nc.sync.dma_start(out=ep, in_=enc_d[seg])
            proj_kv_seg(KT_c, V_c, seg, ep, wkc, wvc, b_sb["bk_c"], "vc")
        x1f = resid.tile([128, FT, TQ], F32, tag="res", bufs=2, name="x1f")
        x1n = resid.tile([128, FT, TQ], BF16, tag="xn", bufs=1, name="x1n")
        ln_apply(st1, x1p, x1f, b_sb["g1"], b_sb["be1"], "ln1", bf16_out=x1n)
        QT_c = glob.tile([128, FT, TQ], BF16, tag="qt", bufs=1, name="QT_c")
        wqc = wtile("wq_c")
        proj_q(QT_c, wqc, x1n, b_sb["bq_c"], "c")

        attnT_c = glob.tile([128, FT, TQ], BF16, tag="attnT", bufs=1,
                            name="attnT_c")
        x2p = resid.tile([128, FT, TQ], F32, tag="res", bufs=2, name="x2p")
        attn_cross(QT_c, KT_c, V_c, attnT_c, b_sb["bv_c"])
        woc = wtile("wo_c")
        wo_resid(attnT_c, woc, b_sb["bo_c"], x1f, x2p)
        st2 = ln_stats(x2p, "ln2")
        x2f = resid.tile([128, FT, TQ], F32, tag="res", bufs=2, name="x2f")
        x2n = resid.tile([128, FT, TQ], BF16, tag="xn", bufs=1, name="x2n")
        ln_apply(st2, x2p, x2f, b_sb["g2"], b_sb["be2"], "ln2", bf16_out=x2n)

        glob_ctx.close()

        # ---- FFN + LN3 + output ----
        x3 = resid.tile([128, FT, TQ], F32, tag="res", bufs=2, name="x3")
        out_sb = resid.tile([128, FT, TQ], F32, tag="res", bufs=2,
                            name="out_sb")
        with ExitStack() as S5:
            fp5 = S5.enter_context(tc.tile_pool(name="ffn", bufs=1))
            h_sb = fp5.tile([128, HT, TQ], BF16, tag="h", bufs=1, name="h_sb")
            # stream W1 in pieces (small first pieces so the first matmul
            # starts as early as possible after SBUF frees up)
            pieces = [2, 2, 4, 8, 8, 8]          # f-tiles per piece
            ht = 0
            for g, npc in enumerate(pieces):
                w1p = fp5.tile([128, FT, npc * 128], BF16, tag="w1", bufs=2,
                               padded_shape=[128, FT, 1024], name=f"w1_{g}")
                nc.sync.dma_start(
                    out=w1p, in_=w1_d[:, :, ht * 128:(ht + npc) * 128])
                for i in range(npc):
                    ps = ps_tile("big" if ht % 2 == 0 else "st", 2,
                                 name=f"pf1_{ht}")
                    for dc in range(FT):
                        nc.tensor.matmul(
                            ps, lhsT=w1p[:, dc, i * 128:(i + 1) * 128],
                            rhs=x2n[:, dc, :],
                            start=(dc == 0), stop=(dc == FT - 1))
                    # bias-add + relu + bf16 cast in one DVE op
                    nc.vector.tensor_scalar(out=h_sb[:, ht, :], in0=ps,
                                            scalar1=b1_sb[:, ht:ht + 1],
                                            scalar2=0.0,
                                            op0=ALU.add, op1=ALU.max)
                    ht += 1
            # W2: ht-outer with 8 concurrent PSUM accumulators (all banks),
            # streaming W2 in 4 pieces.
            tag8 = ["big", "big", "st", "st", "av", "av", "t", "misc"]
            ps8 = [ps_tile(tag8[fo], 2 if fo < 6 else 1, name=f"pf2_{fo}")
                   for fo in range(FT)]
            for g in range(4):
                w2p = fp5.tile([128, FT, D], BF16, tag="w2p", bufs=2,
                               name=f"w2_{g}")
                nc.sync.dma_start(out=w2p, in_=w2_d[:, g * 8:(g + 1) * 8, :])
                for i in range(8):
                    ht = g * 8 + i
                    for fo in range(FT):
                        nc.tensor.matmul(
                            ps8[fo], lhsT=w2p[:, i, fo * 128:(fo + 1) * 128],
                            rhs=h_sb[:, ht, :],
                            start=(ht == 0), stop=(ht == HT - 1))
            for fo in range(FT):
                nc.vector.scalar_tensor_tensor(
                    out=x3[:, fo, :], in0=ps8[fo],
                    scalar=b_sb["b2"][:, fo:fo + 1],
                    in1=x2f[:, fo, :], op0=ALU.add, op1=ALU.add)
            st3 = ln_stats(x3, "ln3")
            ln_apply(st3, x3, out_sb, b_sb["g3"], b_sb["be3"], "ln3",
                     dma_out=out_d)

    nc.compile()
    return nc


def _to_tiles(a2d, dt=ml_dtypes.bfloat16):
    """[P*128, F] -> [128, P, F] (SBUF tile layout), casting to dt."""
    p8, f = a2d.shape
    return np.ascontiguousarray(
        a2d.reshape(p8 // 128, 128, f).transpose(1, 0, 2).astype(dt))


def _seg_tiles(a2d):
    """[1024, NSEG*256] -> [NSEG, 128, 8, 256] bf16 (seg-major tiles)."""
    segs = [_to_tiles(a2d[:, s * 256:(s + 1) * 256]) for s in range(NSEG)]
    return np.ascontiguousarray(np.stack(segs))


def _vec_tiles(v, dt=np.float32):
    """[n*128] -> [128, n]"""
    return np.ascontiguousarray(v.reshape(-1, 128).T.astype(dt))


def _prep_core(c, dec, enc, consts):
    j = c % 4
    b = c // 4
    ja, jb = j, 7 - j
    rest = [ch for ch in range(0, jb) if ch != ja]
    qtok = np.r_[ja * CHUNK:(ja + 1) * CHUNK, jb * CHUNK:(jb + 1) * CHUNK]
    kvtok = np.concatenate(
        [qtok] + [np.arange(ch * CHUNK, (ch + 1) * CHUNK) for ch in rest])
    xq = dec[b][qtok]                       # [512, D]
    xkv = np.zeros((KV, D), np.float32)
    xkv[: len(kvtok)] = dec[b][kvtok]
    real_blocks = len(kvtok) // CHUNK

    # per-256-block additive exp biases (0 = attend, NEG = masked)
    biasa = np.full(8, NEG, np.float32)
    biasa[0] = 0.0                          # own diagonal block
    biasa[2:2 + ja] = 0.0                   # prior chunks in the window
    biasb = np.full(8, NEG, np.float32)
    biasb[:real_blocks] = 0.0

    m = dict(consts)
    m["xq"] = _to_tiles(xq.T)
    m["xres"] = _to_tiles(xq.T, np.float32)
    m["xkv"] = _seg_tiles(xkv.T)
    m["enc"] = _seg_tiles(enc[b].T)
    m["smallf"] = np.ascontiguousarray(np.concatenate(
        [m.pop("smallf_base"),
         np.repeat(biasa[None, :], 128, axis=0),
         np.repeat(biasb[None, :], 128, axis=0)], axis=1, dtype=np.float32))
    return m, (b, qtok)


def _prep_consts(inputs):
    bf = ml_dtypes.bfloat16
    c = {}
    for src, dst in (("Wq_s", "wq_s"), ("Wk_s", "wk_s"), ("Wv_s", "wv_s"),
                     ("Wq_c", "wq_c"), ("Wk_c", "wk_c"), ("Wv_c", "wv_c")):
        w = np.asarray(inputs[src], np.float32)           # [H, D, DK]
        c[dst] = _to_tiles(w.transpose(1, 0, 2).reshape(D, D))
    c["wo_s"] = _to_tiles(np.asarray(inputs["Wo_s"], np.float32))
    c["wo_c"] = _to_tiles(np.asarray(inputs["Wo_c"], np.float32))
    c["w1"] = _to_tiles(np.asarray(inputs["W1"], np.float32))
    c["w2"] = _to_tiles(np.asarray(inputs["W2"], np.float32))
    smalls = [_vec_tiles(np.asarray(inputs[nm], np.float32).reshape(-1))
              for nm in ("bq_s", "bk_s", "bv_s", "bo_s", "bq_c", "bk_c",
                         "bv_c", "bo_c", "b2", "g1", "be1", "g2", "be2",
                         "g3", "be3")]
    smalls.append(_vec_tiles(np.asarray(inputs["b1"], np.float32)))
    c["smallf_base"] = np.concatenate(smalls, axis=1)
    # causal diag mask M[s, q] = 1 if s <= q, packed [128, 512]
    M = (np.arange(CHUNK)[:, None] <= np.arange(CHUNK)[None, :]).astype(bf)
    c["dmask"] = np.ascontiguousarray(
        np.concatenate([M[0:128], M[128:256]], axis=1))
    return c


def kernel(**inputs):
    global _BUILT
    if _BUILT is None:
        _BUILT = _build()
    nc = _BUILT

    dec = np.asarray(inputs["dec_input"], np.float32)
    enc = np.asarray(inputs["enc_output"], np.float32)
    consts = _prep_consts(inputs)
    in_maps = []
    metas = []
    for cix in range(N_CORES):
        m, meta = _prep_core(cix, dec, enc, consts)
        in_maps.append(m)
        metas.append(meta)

    res = run_bass_kernel_spmd(nc, in_maps, core_ids=list(range(N_CORES)))

    out = np.empty((B, T, D), np.float32)
    for cix in range(N_CORES):
        b, qtok = metas[cix]
        tiles = res.results[cix]["out"]       # [128, FT, TQ]
        core_t = tiles.transpose(1, 0, 2).reshape(D, TQ)
        out[b, qtok, :] = core_t.T
    return out


# revision 45
# speedup vs baseline: 1.1148x; 1.0058x over previous
"""Transformer decoder layer (self-attn + cross-attn + FFN, post-LN) on 8
Trainium2 NeuronCores, sequence-parallel with zero collectives.

Sharding: core c -> batch b = c//4, causal-balanced chunk pair (j, 7-j) of
256 tokens each (j = c%4), so every core owns 512 query tokens with equal
total causal attention area. Weights are replicated; K/V projections are
recomputed per core. All per-core differences are expressed through input
DATA (token reordering + additive exp-bias masks), so a single SPMD program
serves all 8 cores.

Layout: activations are kept feature-major [d_partition, token_free] so
every matmul contracts along SBUF partitions with weights as the stationary
operand. Scores are computed transposed (S^T = [s, q]) which makes softmax
need no cross-partition reduction: exp on ScalarE (scores are O(0.5), so no
max subtraction), denominator via an extra ones-column appended to V.
Matmuls run in bf16 with fp32 PSUM accumulation; the residual stream and
layernorm arithmetic stay fp32 (stat sums in bf16, mean/rstd broadcasts via
fp32 matmuls).

Overlap structure: chunk-A attention (which only needs the first 1280 kv
tokens) is emitted mid-way through the K/V projection segments; the whole
cross K/V projection is sandwiched between LN1's stats and apply so the LN
latency chain hides under projection matmuls.
"""

import sys

if "/opt/trn_rl_repo" not in sys.path:
    sys.path.insert(0, "/opt/trn_rl_repo")

from contextlib import ExitStack

import numpy as np
import ml_dtypes

import concourse.bass as bass
import concourse.bacc as bacc
import concourse.tile as tile
import concourse.mybir as mybir
from concourse.bass_utils import run_bass_kernel_spmd
from concourse.masks import make_identity

F32 = mybir.dt.float32
BF16 = mybir.dt.bfloat16
AF = mybir.ActivationFunctionType
ALU = mybir.AluOpType

D = 1024
H = 16
DK = 64
DFF = 4096
B = 2
T = 2048
N_CORES = 8
CHUNK = 256
TQ = 512          # query tokens per core
KV = 2048         # padded kv layout length (self), enc length (cross)
FT = D // 128     # 8 f-tiles
HT = DFF // 128   # 32 ffn tiles
NSEG = 8          # kv/enc DMA-streaming segments of 256 tokens
NEG = -50.0       # additive pre-exp mask (exp(-50) ~ 2e-22)

# self-attn 256-token s-block schedules over the kv layout
# [A(256) | B(256) | rest... | pad]  (block = 2 s-tiles of 128):
BLOCKS_A = [0, 2, 3, 4]    # own diag + 768-token prior window
BLOCKS_B = list(range(8))  # everything (pads masked via bias)

_BUILT = None


def _build():
    nc = bacc.Bacc("TRN2", target_bir_lowering=False, debug=False,
                   num_devices=N_CORES)

    def din(name, shape, dt):
        return nc.dram_tensor(name, shape, dt, kind="ExternalInput").ap()

    xq_d = din("xq", [128, FT, TQ], BF16)
    xres_d = din("xres", [128, FT, TQ], F32)
    xkv_d = din("xkv", [NSEG, 128, FT, 256], BF16)    # seg-major
    enc_d = din("enc", [NSEG, 128, FT, 256], BF16)    # seg-major
    w_d = {}
    for nm in ("wq_s", "wk_s", "wv_s", "wo_s", "wq_c", "wk_c", "wv_c", "wo_c"):
        w_d[nm] = din(nm, [128, FT, D], BF16)
    w1_d = din("w1", [128, FT, DFF], BF16)
    w2_d = din("w2", [128, HT, D], BF16)
    # all small fp32 vectors packed into one tensor: 15 biases/ln params of
    # [128, 8], then b1 [128, 32], biasa2 [128, 8], biasb2 [128, 8]
    BIAS_NAMES = ("bq_s", "bk_s", "bv_s", "bo_s", "bq_c", "bk_c", "bv_c",
                  "bo_c", "b2", "g1", "be1", "g2", "be2", "g3", "be3")
    smallf_d = din("smallf", [128, 15 * FT + HT + 16], F32)
    dmask_d = din("dmask", [128, 512], BF16)
    out_d = nc.dram_tensor("out", [128, FT, TQ], F32, kind="ExternalOutput").ap()

    with tile.TileContext(nc) as tc, ExitStack() as S:
        const = S.enter_context(tc.tile_pool(name="const", bufs=1))
        pp = S.enter_context(tc.tile_pool(name="ps", bufs=1, space="PSUM"))
        resid = S.enter_context(tc.tile_pool(name="resid", bufs=1))

        ident = const.tile([128, 128], BF16)
        make_identity(nc, ident)
        ones_b = const.tile([128, 1], BF16)
        nc.vector.memset(ones_b, 1.0)
        ones_row = const.tile([1, 128], F32)
        nc.vector.memset(ones_row, 1.0)
        eps_t = const.tile([1, 1], F32)
        nc.vector.memset(eps_t, 1e-5)

        glob_ctx = ExitStack()
        glob = glob_ctx.enter_context(tc.tile_pool(name="glob", bufs=1))

        # =========== helpers ===========
        def ps_tile(tag, bufs, shape=(128, 512), dt=F32, name="ps"):
            return pp.tile(list(shape), dt, tag=tag, bufs=bufs, name=name)

        def wtile(nm):
            t = glob.tile([128, FT, D], BF16, tag="wstream", bufs=2, name=nm)
            for dc in range(FT):     # per-chunk so first consumers start early
                nc.sync.dma_start(out=t[:, dc, :], in_=w_d[nm][:, dc, :])
            return t

        TAG8 = ["big", "big", "st", "st", "av", "av", "t", "misc"]
        BUF8 = [2, 2, 2, 2, 2, 2, 1, 1]

        def proj_q(out_t, W_sb, X_sb, bias_t, lbl):
            # dc-outer with 8 concurrent accumulators: the first matmul only
            # needs the first d-chunk of W and X (fast start after DMA).
            ps8 = [ps_tile(TAG8[ft], BUF8[ft], name=f"pjq_{lbl}_{ft}")
                   for ft in range(FT)]
            for dc in range(FT):
                for ft in range(FT):
                    nc.tensor.matmul(
                        ps8[ft], lhsT=W_sb[:, dc, ft * 128:(ft + 1) * 128],
                        rhs=X_sb[:, dc, :],
                        start=(dc == 0), stop=(dc == FT - 1))
            for ft in range(FT):
                nc.scalar.activation(out=out_t[:, ft, :], in_=ps8[ft],
                                     func=AF.Identity,
                                     bias=bias_t[:, ft:ft + 1], scale=1.0)

        def proj_kv_seg(KT, V_list, seg, X_piece, WK_sb, WV_sb, bk_t, vtag):
            """one 256-token segment of V (token-major) then K^T (f-major)."""
            sl = slice(seg * 256, (seg + 1) * 256)
            for sti in range(2):
                st = seg * 2 + sti
                vt = glob.tile([128, H, DK + 1], BF16, tag="v", bufs=16,
                               name=f"v_{vtag}_{st}")
                for half in range(2):
                    ps = ps_tile("big" if half == 0 else "st", 2,
                                 name=f"pv_{vtag}_{st}_{half}")
                    for dc in range(FT):
                        nc.tensor.matmul(
                            ps,
                            lhsT=X_piece[:, dc, sti * 128:(sti + 1) * 128],
                            rhs=WV_sb[:, dc, half * 512:(half + 1) * 512],
                            start=(dc == 0), stop=(dc == FT - 1))
                    nc.vector.tensor_copy(
                        out=vt[:, half * 8:(half + 1) * 8, 0:DK],
                        in_=ps.rearrange("p (a b) -> p a b", b=DK))
                nc.vector.memset(vt[:, :, DK:DK + 1], 1.0)
                V_list.append(vt)
            for ft in range(FT):
                ps = ps_tile("big" if ft % 2 == 0 else "st", 2,
                             shape=(128, 256), name=f"pjk_{vtag}_{seg}_{ft}")
                for dc in range(FT):
                    nc.tensor.matmul(
                        ps, lhsT=WK_sb[:, dc, ft * 128:(ft + 1) * 128],
                        rhs=X_piece[:, dc, :],
                        start=(dc == 0), stop=(dc == FT - 1))
                nc.vector.tensor_scalar_add(out=KT[:, ft, sl], in0=ps,
                                            scalar1=bk_t[:, ft:ft + 1])

        # Normalized attention tiles go through a PE transpose whose input
        # comes from a short DVE chain; emitting the transpose immediately
        # would stall the in-order PE stream on DVE. Instead stage-1 (DVE
        # recip+scale) is emitted with the AV matmuls and the transposes are
        # deferred into the NEXT head's PE stream.
        pending_t = []
        _tcnt = [0]

        def _norm1(psav, attnT, po, fp, q0, bv_t, nm):
            rec = glob.tile([128, 1], F32, tag="rec", bufs=10, name=f"r{nm}")
            nc.vector.reciprocal(rec, psav[:, DK:DK + 1])
            an = glob.tile([128, DK], BF16, tag="an", bufs=10, name=f"n{nm}")
            nc.vector.tensor_scalar_mul(an, psav[:, 0:DK], rec)
            pending_t.append((an, attnT, po, fp, q0, bv_t))

        def flush_t():
            for an, attnT, po, fp, q0, bv_t in pending_t:
                _tcnt[0] += 1
                pst = ps_tile("t" if _tcnt[0] % 2 == 0 else "misc", 1,
                              shape=(DK, 128), dt=BF16, name=f"pt{_tcnt[0]}")
                nc.tensor.transpose(pst, an, ident)
                nc.vector.tensor_scalar_add(
                    out=attnT[po:po + DK, fp, q0:q0 + 128], in0=pst,
                    scalar1=bv_t[po:po + DK, fp:fp + 1])
            pending_t.clear()

        def attn_chunk(QT, KT, V_list, attnT, bv_t, cn, qoff, blocks, bias2,
                       diag_blk):
            for h in range(H):
                fp, po = h // 2, (h % 2) * DK
                ats = {}
                for blk in blocks:
                    ps = ps_tile("st", 2, name=f"pss_{h}_{cn}_{blk}")
                    for half in range(2):
                        st = blk * 2 + half
                        nc.tensor.matmul(
                            ps[:, half * 256:(half + 1) * 256],
                            lhsT=KT[po:po + DK, fp, st * 128:(st + 1) * 128],
                            rhs=QT[po:po + DK, fp, qoff:qoff + CHUNK],
                            start=True, stop=True)
                    at = glob.tile([128, 512], BF16, tag="at", bufs=16,
                                   name=f"a_{h}_{cn}_{blk}")
                    nc.scalar.activation(out=at, in_=ps, func=AF.Exp,
                                         scale=0.125,
                                         bias=bias2[:, blk:blk + 1])
                    if blk == diag_blk:
                        nc.vector.tensor_mul(at, at, dmask_sb)
                    ats[blk] = at
                flush_t()
                for qt in range(2):
                    psav = ps_tile("av" if qt % 2 == 0 else "big", 2,
                                   shape=(128, DK + 1),
                                   name=f"pav_{h}_{cn}_{qt}")
                    units = [(blk, half) for blk in blocks
                             for half in range(2)]
                    for i, (blk, half) in enumerate(units):
                        st = blk * 2 + half
                        nc.tensor.matmul(
                            psav,
                            lhsT=ats[blk][:, half * 256 + qt * 128:
                                          half * 256 + (qt + 1) * 128],
                            rhs=V_list[st][:, h, :],
                            start=(i == 0), stop=(i == len(units) - 1))
                    _norm1(psav, attnT, po, fp, qoff + qt * 128, bv_t,
                           f"s_{h}_{cn}_{qt}")
            flush_t()

        def attn_cross(QT, KT, V_list, attnT, bv_t):
            # two half-passes over s so only 8 exp tiles are live per head
            for h in range(H):
                fp, po = h // 2, (h % 2) * DK
                psavs = [ps_tile("av" if qt % 2 == 0 else "big", 2,
                                 shape=(128, DK + 1), name=f"pavc_{h}_{qt}")
                         for qt in range(4)]
                for half in range(2):
                    ats = {}
                    for st in range(half * 8, half * 8 + 8):
                        ps = ps_tile("st", 2, name=f"psc_{h}_{st}")
                        nc.tensor.matmul(
                            ps,
                            lhsT=KT[po:po + DK, fp, st * 128:(st + 1) * 128],
                            rhs=QT[po:po + DK, fp, :], start=True, stop=True)
                        at = glob.tile([128, 512], BF16, tag="at", bufs=16,
                                       name=f"ac_{h}_{st}")
                        nc.scalar.activation(out=at, in_=ps, func=AF.Exp,
                                             scale=0.125)
                        ats[st] = at
                    if half == 0:
                        flush_t()
                    for qt in range(4):
                        for st in range(half * 8, half * 8 + 8):
                            nc.tensor.matmul(
                                psavs[qt],
                                lhsT=ats[st][:, qt * 128:(qt + 1) * 128],
                                rhs=V_list[st][:, h, :],
                                start=(st == 0), stop=(st == 15))
                for qt in range(4):
                    _norm1(psavs[qt], attnT, po, fp, qt * 128, bv_t,
                           f"c_{h}_{qt}")
            flush_t()

        def wo_resid(attnT, WO_sb, bo_t, x_prev, x_out):
            for fo in range(FT):
                ps = ps_tile("big" if fo % 2 == 0 else "st", 2,
                             name=f"pwo_{fo}")
                for fi in range(FT):
                    nc.tensor.matmul(ps,
                                     lhsT=WO_sb[:, fi, fo * 128:(fo + 1) * 128],
                                     rhs=attnT[:, fi, :],
                                     start=(fi == 0), stop=(fi == FT - 1))
                nc.vector.scalar_tensor_tensor(
                    out=x_out[:, fo, :], in0=ps, scalar=bo_t[:, fo:fo + 1],
                    in1=x_prev[:, fo, :], op0=ALU.add, op1=ALU.add)

        def ln_stats(x_in, lbl):
            """-> (ps_mu, ps_rstd) broadcast PSUM tiles (tags misc/t)."""
            ps_sum = ps_tile("st", 2, shape=(1, TQ), name=f"psum_{lbl}")
            ps_sq = ps_tile("big", 2, shape=(1, TQ), name=f"psq_{lbl}")
            for fc in range(FT):
                xb = resid.tile([128, TQ], BF16, tag="sqb", bufs=3,
                                name=f"xb_{lbl}_{fc}")
                nc.vector.tensor_copy(out=xb, in_=x_in[:, fc, :])
                nc.tensor.matmul(ps_sum, lhsT=ones_b, rhs=xb,
                                 start=(fc == 0), stop=(fc == FT - 1))
                sqb = resid.tile([128, TQ], BF16, tag="sqb", bufs=3,
                                 name=f"sq_{lbl}_{fc}")
                nc.vector.tensor_mul(sqb, xb, xb)
                nc.tensor.matmul(ps_sq, lhsT=ones_b, rhs=sqb,
                                 start=(fc == 0), stop=(fc == FT - 1))
            mu = resid.tile([1, TQ], F32, tag="stat", bufs=2, name=f"mu_{lbl}")
            nc.scalar.activation(out=mu, in_=ps_sum, func=AF.Copy, scale=1.0 / D)
            msq = resid.tile([1, TQ], F32, tag="stat", bufs=2,
                             name=f"msq_{lbl}")
            nc.scalar.activation(out=msq, in_=ps_sq, func=AF.Copy, scale=1.0 / D)
            mu2 = resid.tile([128, TQ], F32, tag="sq", bufs=2,
                             name=f"mu2_{lbl}")
            nc.vector.tensor_mul(mu2[0:1, :], mu, mu)
            nc.vector.tensor_sub(msq, msq, mu2[0:1, :])  # msq <- var
            nc.scalar.activation(out=msq, in_=msq, func=AF.Sqrt, bias=eps_t,
                                 scale=1.0)              # msq <- std
            ps_mu = ps_tile("misc", 1, name=f"pmu_{lbl}")
            nc.tensor.matmul(ps_mu, lhsT=ones_row, rhs=mu, start=True,
                             stop=True)
            rstd = resid.tile([1, TQ], F32, tag="stat", bufs=2,
                              name=f"rstd_{lbl}")
            nc.vector.reciprocal(rstd, msq)
            ps_rstd = ps_tile("t", 1, name=f"prstd_{lbl}")
            nc.tensor.matmul(ps_rstd, lhsT=ones_row, rhs=rstd, start=True,
                             stop=True)
            return ps_mu, ps_rstd

        def ln_apply(stats, x_in, out_t, g_t, be_t, lbl, dma_out=None,
                     bf16_out=None):
            ps_mu, ps_rstd = stats
            for fc in range(FT):
                tmp = resid.tile([128, TQ], F32, tag="sq", bufs=2,
                                 name=f"t_{lbl}_{fc}")
                nc.vector.tensor_sub(tmp, x_in[:, fc, :], ps_mu)
                nc.vector.tensor_mul(tmp, tmp, ps_rstd)
                nc.vector.tensor_scalar(out=out_t[:, fc, :], in0=tmp,
                                        scalar1=g_t[:, fc:fc + 1],
                                        scalar2=be_t[:, fc:fc + 1],
                                        op0=ALU.mult, op1=ALU.add)
                if bf16_out is not None:
                    nc.vector.tensor_copy(out=bf16_out[:, fc, :],
                                          in_=out_t[:, fc, :])
                if dma_out is not None:
                    nc.sync.dma_start(out=dma_out[:, fc, :],
                                      in_=out_t[:, fc, :])

        # =========== program ===========
        QT = glob.tile([128, FT, TQ], BF16, tag="qt", bufs=1, name="QT_s")
        KT = glob.tile([128, FT, KV], BF16, tag="kt", bufs=1, name="KT_s")
        attnT = glob.tile([128, FT, TQ], BF16, tag="attnT", bufs=1,
                          name="attnT_s")
        V_s = []
        x_res = resid.tile([128, FT, TQ], F32, tag="res", bufs=2)
        x1p = resid.tile([128, FT, TQ], F32, tag="res", bufs=2, name="x1p")
        with ExitStack() as S1:
            wp = S1.enter_context(tc.tile_pool(name="wself", bufs=1))
            xq_b = wp.tile([128, FT, TQ], BF16, tag="xq", bufs=1)
            wq = glob.tile([128, FT, D], BF16, tag="wstream", bufs=2,
                           name="wq_s")
            for dc in range(FT):   # per-chunk loads so compute starts early
                nc.sync.dma_start(out=xq_b[:, dc, :], in_=xq_d[:, dc, :])
                nc.sync.dma_start(out=wq[:, dc, :], in_=w_d["wq_s"][:, dc, :])
            # small consts: one packed DMA on the gpsimd queue
            smallf = const.tile([128, 15 * FT + HT + 16], F32, name="c_small")
            nc.gpsimd.dma_start(out=smallf, in_=smallf_d)
            dmask_sb = const.tile([128, 512], BF16, name="c_dm")
            nc.gpsimd.dma_start(out=dmask_sb, in_=dmask_d)
            b_sb = {nm: smallf[:, i * FT:(i + 1) * FT]
                    for i, nm in enumerate(BIAS_NAMES)}
            b1_sb = smallf[:, 15 * FT:15 * FT + HT]
            biasa_sb = smallf[:, 15 * FT + HT:15 * FT + HT + 8]
            biasb_sb = smallf[:, 15 * FT + HT + 8:15 * FT + HT + 16]

            xp0 = wp.tile([128, FT, 256], BF16, tag="xkvp", bufs=2,
                          name="xkv_0")
            nc.sync.dma_start(out=xp0, in_=xkv_d[0])
            proj_q(QT, wq, xq_b, b_sb["bq_s"], "s")
            wv = wtile("wv_s")
            wk = wtile("wk_s")
            proj_kv_seg(KT, V_s, 0, xp0, wk, wv, b_sb["bk_s"], "v")
            nc.sync.dma_start(out=x_res, in_=xres_d)
            for seg in range(1, 5):
                xp = wp.tile([128, FT, 256], BF16, tag="xkvp", bufs=2,
                             name=f"xkv_{seg}")
                nc.sync.dma_start(out=xp, in_=xkv_d[seg])
                proj_kv_seg(KT, V_s, seg, xp, wk, wv, b_sb["bk_s"], "v")
            # chunk-A attention only needs kv tiles 0..9 (segs 0..4)
            attn_chunk(QT, KT, V_s, attnT, b_sb["bv_s"], "A", 0, BLOCKS_A,
                       biasa_sb, 0)
            for seg in range(5, NSEG):
                xp = wp.tile([128, FT, 256], BF16, tag="xkvp", bufs=2,
                             name=f"xkv_{seg}")
                nc.sync.dma_start(out=xp, in_=xkv_d[seg])
                proj_kv_seg(KT, V_s, seg, xp, wk, wv, b_sb["bk_s"], "v")

        attn_chunk(QT, KT, V_s, attnT, b_sb["bv_s"], "B", CHUNK, BLOCKS_B,
                   biasb_sb, 1)
        wo = wtile("wo_s")
        wo_resid(attnT, wo, b_sb["bo_s"], x_res, x1p)

        # LN1 stats now; the whole cross K/V projection runs while the
        # mean/rstd chain resolves; LN1 apply afterwards.
        st1 = ln_stats(x1p, "ln1")
        KT_c = glob.tile([128, FT, KV], BF16, tag="kt", bufs=1, name="KT_c")
        V_c = []
        wvc = wtile("wv_c")
        wkc = wtile("wk_c")
        for seg in range(NSEG):
            ep = glob.tile([128, FT, 256], BF16, tag="encp", bufs=2,
                           name=f"enc_{seg}")
            nc.sync.dma_start(out=ep, in_=enc_d[seg])
            proj_kv_seg(KT_c, V_c, seg, ep, wkc, wvc, b_sb["bk_c"], "vc")
        x1f = resid.tile([128, FT, TQ], F32, tag="res", bufs=2, name="x1f")
        x1n = resid.tile([128, FT, TQ], BF16, tag="xn", bufs=1, name="x1n")
        ln_apply(st1, x1p, x1f, b_sb["g1"], b_sb["be1"], "ln1", bf16_out=x1n)
        QT_c = glob.tile([128, FT, TQ], BF16, tag="qt", bufs=1, name="QT_c")
        wqc = wtile("wq_c")
        proj_q(QT_c, wqc, x1n, b_sb["bq_c"], "c")

        attnT_c = glob.tile([128, FT, TQ], BF16, tag="attnT", bufs=1,
                            name="attnT_c")
        x2p = resid.tile([128, FT, TQ], F32, tag="res", bufs=2, name="x2p")
        attn_cross(QT_c, KT_c, V_c, attnT_c, b_sb["bv_c"])
        woc = wtile("wo_c")
        wo_resid(attnT_c, woc, b_sb["bo_c"], x1f, x2p)
        st2 = ln_stats(x2p, "ln2")
        x2f = resid.tile([128, FT, TQ], F32, tag="res", bufs=2, name="x2f")
        x2n = resid.tile([128, FT, TQ], BF16, tag="xn", bufs=1, name="x2n")
        ln_apply(st2, x2p, x2f, b_sb["g2"], b_sb["be2"], "ln2", bf16_out=x2n)

        glob_ctx.close()

        # ---- FFN + LN3 + output ----
        x3 = resid.tile([128, FT, TQ], F32, tag="res", bufs=2, name="x3")
        out_sb = resid.tile([128, FT, TQ], F32, tag="res", bufs=2,
                            name="out_sb")
        with ExitStack() as S5:
            fp5 = S5.enter_context(tc.tile_pool(name="ffn", bufs=1))
            h_sb = fp5.tile([128, HT, TQ], BF16, tag="h", bufs=1, name="h_sb")
            # stream W1 in pieces (small first pieces so the first matmul
            # starts as early as possible after SBUF frees up)
            pieces = [2, 2, 4, 8, 8, 8]          # f-tiles per piece
            ht = 0
            for g, npc in enumerate(pieces):
                w1p = fp5.tile([128, FT, npc * 128], BF16, tag="w1", bufs=2,
                               padded_shape=[128, FT, 1024], name=f"w1_{g}")
                nc.sync.dma_start(
                    out=w1p, in_=w1_d[:, :, ht * 128:(ht + npc) * 128])
                for i in range(npc):
                    ps = ps_tile("big" if ht % 2 == 0 else "st", 2,
                                 name=f"pf1_{ht}")
                    for dc in range(FT):
                        nc.tensor.matmul(
                            ps, lhsT=w1p[:, dc, i * 128:(i + 1) * 128],
                            rhs=x2n[:, dc, :],
                            start=(dc == 0), stop=(dc == FT - 1))
                    # bias-add + relu + bf16 cast in one DVE op
                    nc.vector.tensor_scalar(out=h_sb[:, ht, :], in0=ps,
                                            scalar1=b1_sb[:, ht:ht + 1],
                                            scalar2=0.0,
                                            op0=ALU.add, op1=ALU.max)
                    ht += 1
            # W2: ht-outer with 8 concurrent PSUM accumulators (all banks),
            # streaming W2 in 4 pieces.
            tag8 = ["big", "big", "st", "st", "av", "av", "t", "misc"]
            ps8 = [ps_tile(tag8[fo], 2 if fo < 6 else 1, name=f"pf2_{fo}")
                   for fo in range(FT)]
            for g in range(4):
                w2p = fp5.tile([128, FT, D], BF16, tag="w2p", bufs=2,
                               name=f"w2_{g}")
                nc.sync.dma_start(out=w2p, in_=w2_d[:, g * 8:(g + 1) * 8, :])
                for i in range(8):
                    ht = g * 8 + i
                    for fo in range(FT):
                        nc.tensor.matmul(
                            ps8[fo], lhsT=w2p[:, i, fo * 128:(fo + 1) * 128],
                            rhs=h_sb[:, ht, :],
                            start=(ht == 0), stop=(ht == HT - 1))
            for fo in range(FT):
                nc.vector.scalar_tensor_tensor(
                    out=x3[:, fo, :], in0=ps8[fo],
                    scalar=b_sb["b2"][:, fo:fo + 1],
                    in1=x2f[:, fo, :], op0=ALU.add, op1=ALU.add)
            st3 = ln_stats(x3, "ln3")
            ln_apply(st3, x3, out_sb, b_sb["g3"], b_sb["be3"], "ln3",
                     dma_out=out_d)

    nc.compile()
    return nc


def _to_tiles(a2d, dt=ml_dtypes.bfloat16):
    """[P*128, F] -> [128, P, F] (SBUF tile layout), casting to dt."""
    p8, f = a2d.shape
    return np.ascontiguousarray(
        a2d.reshape(p8 // 128, 128, f).transpose(1, 0, 2).astype(dt))


def _seg_tiles(a2d):
    """[1024, NSEG*256] -> [NSEG, 128, 8, 256] bf16 (seg-major tiles)."""
    segs = [_to_tiles(a2d[:, s * 256:(s + 1) * 256]) for s in range(NSEG)]
    return np.ascontiguousarray(np.stack(segs))


def _vec_tiles(v, dt=np.float32):
    """[n*128] -> [128, n]"""
    return np.ascontiguousarray(v.reshape(-1, 128).T.astype(dt))


def _prep_core(c, dec, enc, consts):
    j = c % 4
    b = c // 4
    ja, jb = j, 7 - j
    rest = [ch for ch in range(0, jb) if ch != ja]
    qtok = np.r_[ja * CHUNK:(ja + 1) * CHUNK, jb * CHUNK:(jb + 1) * CHUNK]
    kvtok = np.concatenate(
        [qtok] + [np.arange(ch * CHUNK, (ch + 1) * CHUNK) for ch in rest])
    xq = dec[b][qtok]                       # [512, D]
    xkv = np.zeros((KV, D), np.float32)
    xkv[: len(kvtok)] = dec[b][kvtok]
    real_blocks = len(kvtok) // CHUNK

    # per-256-block additive exp biases (0 = attend, NEG = masked)
    biasa = np.full(8, NEG, np.float32)
    biasa[0] = 0.0                          # own diagonal block
    biasa[2:2 + ja] = 0.0                   # prior chunks in the window
    biasb = np.full(8, NEG, np.float32)
    biasb[:real_blocks] = 0.0

    m = dict(consts)
    m["xq"] = _to_tiles(xq.T)
    m["xres"] = _to_tiles(xq.T, np.float32)
    m["xkv"] = _seg_tiles(xkv.T)
    m["enc"] = _seg_tiles(enc[b].T)
    m["smallf"] = np.ascontiguousarray(np.concatenate(
        [m.pop("smallf_base"),
         np.repeat(biasa[None, :], 128, axis=0),
         np.repeat(biasb[None, :], 128, axis=0)], axis=1, dtype=np.float32))
    return m, (b, qtok)


def _prep_consts(inputs):
    bf = ml_dtypes.bfloat16
    c = {}
    for src, dst in (("Wq_s", "wq_s"), ("Wk_s", "wk_s"), ("Wv_s", "wv_s"),
                     ("Wq_c", "wq_c"), ("Wk_c", "wk_c"), ("Wv_c", "wv_c")):
        w = np.asarray(inputs[src], np.float32)           # [H, D, DK]
        c[dst] = _to_tiles(w.transpose(1, 0, 2).reshape(D, D))
    c["wo_s"] = _to_tiles(np.asarray(inputs["Wo_s"], np.float32))
    c["wo_c"] = _to_tiles(np.asarray(inputs["Wo_c"], np.float32))
    c["w1"] = _to_tiles(np.asarray(inputs["W1"], np.float32))
    c["w2"] = _to_tiles(np.asarray(inputs["W2"], np.float32))
    smalls = [_vec_tiles(np.asarray(inputs[nm], np.float32).reshape(-1))
              for nm in ("bq_s", "bk_s", "bv_s", "bo_s", "bq_c", "bk_c",
                         "bv_c", "bo_c", "b2", "g1", "be1", "g2", "be2",
                         "g3", "be3")]
    smalls.append(_vec_tiles(np.asarray(inputs["b1"], np.float32)))
    c["smallf_base"] = np.concatenate(smalls, axis=1)
    # causal diag mask M[s, q] = 1 if s <= q, packed [128, 512]
    M = (np.arange(CHUNK)[:, None] <= np.arange(CHUNK)[None, :]).astype(bf)
    c["dmask"] = np.ascontiguousarray(
        np.concatenate([M[0:128], M[128:256]], axis=1))
    return c


def _make_runner(nc):
    """Build the shard_map-jitted executable ONCE (run_bass_kernel_spmd
    re-traces and re-lowers per call, which costs seconds of host time)."""
    import jax
    import concourse.mybir as mybir_
    from concourse import bass2jax
    from jax.experimental.shard_map import shard_map
    from jax.sharding import Mesh, PartitionSpec

    bass2jax.install_neuronx_cc_hook()
    part_name = (nc.partition_id_tensor.name if nc.partition_id_tensor
                 else None)
    in_names, out_names, out_avals, zero_outs = [], [], [], []
    for alloc in nc.m.functions[0].allocations:
        if not isinstance(alloc, mybir_.MemoryLocationSet):
            continue
        name = alloc.memorylocations[0].name
        if alloc.kind == "ExternalInput":
            if name != part_name:
                in_names.append(name)
        elif alloc.kind == "ExternalOutput":
            shape = tuple(alloc.tensor_shape)
            dtype = mybir_.dt.np(alloc.dtype)
            out_names.append(name)
            out_avals.append(jax.core.ShapedArray(shape, dtype))
            zero_outs.append(np.zeros(shape, dtype))
    n_params = len(in_names)
    all_names = in_names + out_names
    if part_name is not None:
        all_names = all_names + [part_name]
    donate = tuple(range(n_params, n_params + len(out_names)))

    def _body(*args):
        operands = list(args)
        if part_name is not None:
            operands.append(bass2jax.partition_id_tensor())
        outs = bass2jax._bass_exec_p.bind(
            *operands, out_avals=tuple(out_avals), in_names=tuple(all_names),
            out_names=tuple(out_names), lowering_input_output_aliases=(),
            sim_require_finite=True, sim_require_nnan=True, nc=nc)
        return tuple(outs)

    devices = jax.devices()[:N_CORES]
    mesh = Mesh(np.asarray(devices), ("core",))
    nin = n_params + len(out_names)
    sharded = jax.jit(
        shard_map(_body, mesh=mesh, in_specs=(PartitionSpec("core"),) * nin,
                  out_specs=(PartitionSpec("core"),) * len(out_names),
                  check_rep=False),
        donate_argnums=donate, keep_unused=True)

    def run(in_maps):
        concat_in = [
            np.concatenate([in_maps[c][nm] for c in range(N_CORES)], axis=0)
            for nm in in_names]
        concat_zero = [
            np.zeros((N_CORES * z.shape[0], *z.shape[1:]), z.dtype)
            for z in zero_outs]
        out_arrs = sharded(*concat_in, *concat_zero)
        return [
            {nm: np.asarray(out_arrs[i]).reshape(N_CORES, *out_avals[i].shape)[c]
             for i, nm in enumerate(out_names)}
            for c in range(N_CORES)]

    return run


def kernel(**inputs):
    global _BUILT
    if _BUILT is None:
        nc = _build()
        _BUILT = _make_runner(nc)
    run = _BUILT

    dec = np.asarray(inputs["dec_input"], np.float32)
    enc = np.asarray(inputs["enc_output"], np.float32)
    consts = _prep_consts(inputs)
    in_maps = []
    metas = []
    for cix in range(N_CORES):
        m, meta = _prep_core(cix, dec, enc, consts)
        in_maps.append(m)
        metas.append(meta)

    results = run(in_maps)

    out = np.empty((B, T, D), np.float32)
    for cix in range(N_CORES):
        b, qtok = metas[cix]
        tiles = results[cix]["out"]           # [128, FT, TQ]
        core_t = tiles.transpose(1, 0, 2).reshape(D, TQ)
        out[b, qtok, :] = core_t.T
    return out


# revision 48
# speedup vs baseline: 1.1881x; 1.0658x over previous
"""Transformer decoder layer (self-attn + cross-attn + FFN, post-LN) on 8
Trainium2 NeuronCores, sequence-parallel with zero collectives.

Sharding: core c -> batch b = c//4, causal-balanced chunk pair (j, 7-j) of
256 tokens each (j = c%4), so every core owns 512 query tokens with equal
total causal attention area. Weights are replicated; K/V projections are
recomputed per core. All per-core differences are expressed through input
DATA (token reordering + additive exp-bias masks), so a single SPMD program
serves all 8 cores.

Layout: activations are kept feature-major [d_partition, token_free] so
every matmul contracts along SBUF partitions with weights as the stationary
operand. Scores are computed transposed (S^T = [s, q]) which makes softmax
need no cross-partition reduction: exp on ScalarE (scores are O(0.5), so no
max subtraction), denominator via an extra ones-column appended to V.
Matmuls run in bf16 with fp32 PSUM accumulation; the residual stream and
layernorm arithmetic stay fp32 (stat sums in bf16, mean/rstd broadcasts via
fp32 matmuls).

Overlap structure: chunk-A attention (which only needs the first 1280 kv
tokens) is emitted mid-way through the K/V projection segments; the whole
cross K/V projection is sandwiched between LN1's stats and apply so the LN
latency chain hides under projection matmuls.
"""

import sys

if "/opt/trn_rl_repo" not in sys.path:
    sys.path.insert(0, "/opt/trn_rl_repo")

from contextlib import ExitStack

import numpy as np
import ml_dtypes

import concourse.bass as bass
import concourse.bacc as bacc
import concourse.tile as tile
import concourse.mybir as mybir
from concourse.bass_utils import run_bass_kernel_spmd
from concourse.masks import make_identity

F32 = mybir.dt.float32
BF16 = mybir.dt.bfloat16
AF = mybir.ActivationFunctionType
ALU = mybir.AluOpType

D = 1024
H = 16
DK = 64
DFF = 4096
B = 2
T = 2048
N_CORES = 8
CHUNK = 256
TQ = 512          # query tokens per core
KV = 2048         # padded kv layout length (self), enc length (cross)
FT = D // 128     # 8 f-tiles
HT = DFF // 128   # 32 ffn tiles
NSEG = 8          # kv/enc DMA-streaming segments of 256 tokens
NEG = -50.0       # additive pre-exp mask (exp(-50) ~ 2e-22)

# self-attn 256-token s-block schedules over the kv layout
# [A(256) | B(256) | rest... | pad]  (block = 2 s-tiles of 128):
BLOCKS_A = [0, 2, 3, 4]    # own diag + 768-token prior window
BLOCKS_B = list(range(8))  # everything (pads masked via bias)

_BUILT = None


def _build():
    nc = bacc.Bacc("TRN2", target_bir_lowering=False, debug=False,
                   num_devices=N_CORES)

    def din(name, shape, dt):
        return nc.dram_tensor(name, shape, dt, kind="ExternalInput").ap()

    xq_d = din("xq", [128, FT, TQ], BF16)
    xres_d = din("xres", [128, FT, TQ], F32)
    xkv_d = din("xkv", [NSEG, 128, FT, 256], BF16)    # seg-major
    enc_d = din("enc", [NSEG, 128, FT, 256], BF16)    # seg-major
    w_d = {}
    for nm in ("wq_s", "wk_s", "wv_s", "wo_s", "wq_c", "wk_c", "wv_c", "wo_c"):
        w_d[nm] = din(nm, [128, FT, D], BF16)
    w1_d = din("w1", [128, FT, DFF], BF16)
    w2_d = din("w2", [128, HT, D], BF16)
    # all small fp32 vectors packed into one tensor: 15 biases/ln params of
    # [128, 8], then b1 [128, 32], biasa2 [128, 8], biasb2 [128, 8]
    BIAS_NAMES = ("bq_s", "bk_s", "bv_s", "bo_s", "bq_c", "bk_c", "bv_c",
                  "bo_c", "b2", "g1", "be1", "g2", "be2", "g3", "be3")
    smallf_d = din("smallf", [128, 15 * FT + HT + 16], F32)
    dmask_d = din("dmask", [128, 512], BF16)
    out_d = nc.dram_tensor("out", [128, FT, TQ], F32, kind="ExternalOutput").ap()

    with tile.TileContext(nc) as tc, ExitStack() as S:
        const = S.enter_context(tc.tile_pool(name="const", bufs=1))
        pp = S.enter_context(tc.tile_pool(name="ps", bufs=1, space="PSUM"))
        resid = S.enter_context(tc.tile_pool(name="resid", bufs=1))

        ident = const.tile([128, 128], BF16)
        make_identity(nc, ident)
        ones_b = const.tile([128, 1], BF16)
        nc.vector.memset(ones_b, 1.0)
        ones_row = const.tile([1, 128], F32)
        nc.vector.memset(ones_row, 1.0)
        eps_t = const.tile([1, 1], F32)
        nc.vector.memset(eps_t, 1e-5)

        glob_ctx = ExitStack()
        glob = glob_ctx.enter_context(tc.tile_pool(name="glob", bufs=1))

        # =========== helpers ===========
        def ps_tile(tag, bufs, shape=(128, 512), dt=F32, name="ps"):
            return pp.tile(list(shape), dt, tag=tag, bufs=bufs, name=name)

        def wtile(nm):
            t = glob.tile([128, FT, D], BF16, tag="wstream", bufs=2, name=nm)
            for dc in range(FT):     # per-chunk so first consumers start early
                nc.sync.dma_start(out=t[:, dc, :], in_=w_d[nm][:, dc, :])
            return t

        TAG8 = ["big", "big", "st", "st", "av", "av", "t", "misc"]
        BUF8 = [2, 2, 2, 2, 2, 2, 1, 1]

        def proj_q(out_t, W_sb, X_sb, bias_t, lbl):
            # dc-outer with 8 concurrent accumulators: the first matmul only
            # needs the first d-chunk of W and X (fast start after DMA).
            ps8 = [ps_tile(TAG8[ft], BUF8[ft], name=f"pjq_{lbl}_{ft}")
                   for ft in range(FT)]
            for dc in range(FT):
                for ft in range(FT):
                    nc.tensor.matmul(
                        ps8[ft], lhsT=W_sb[:, dc, ft * 128:(ft + 1) * 128],
                        rhs=X_sb[:, dc, :],
                        start=(dc == 0), stop=(dc == FT - 1))
            for ft in range(FT):
                nc.scalar.activation(out=out_t[:, ft, :], in_=ps8[ft],
                                     func=AF.Identity,
                                     bias=bias_t[:, ft:ft + 1], scale=1.0)

        def proj_kv_seg(KT, V_list, seg, X_piece, WK_sb, WV_sb, bk_t, vtag):
            """one 256-token segment of V (token-major) then K^T (f-major)."""
            sl = slice(seg * 256, (seg + 1) * 256)
            for sti in range(2):
                st = seg * 2 + sti
                vt = glob.tile([128, H, DK + 1], BF16, tag="v", bufs=16,
                               name=f"v_{vtag}_{st}")
                for half in range(2):
                    ps = ps_tile("big" if half == 0 else "st", 2,
                                 name=f"pv_{vtag}_{st}_{half}")
                    for dc in range(FT):
                        nc.tensor.matmul(
                            ps,
                            lhsT=X_piece[:, dc, sti * 128:(sti + 1) * 128],
                            rhs=WV_sb[:, dc, half * 512:(half + 1) * 512],
                            start=(dc == 0), stop=(dc == FT - 1))
                    nc.vector.tensor_copy(
                        out=vt[:, half * 8:(half + 1) * 8, 0:DK],
                        in_=ps.rearrange("p (a b) -> p a b", b=DK))
                nc.vector.memset(vt[:, :, DK:DK + 1], 1.0)
                V_list.append(vt)
            for ft in range(FT):
                ps = ps_tile("big" if ft % 2 == 0 else "st", 2,
                             shape=(128, 256), name=f"pjk_{vtag}_{seg}_{ft}")
                for dc in range(FT):
                    nc.tensor.matmul(
                        ps, lhsT=WK_sb[:, dc, ft * 128:(ft + 1) * 128],
                        rhs=X_piece[:, dc, :],
                        start=(dc == 0), stop=(dc == FT - 1))
                nc.vector.tensor_scalar_add(out=KT[:, ft, sl], in0=ps,
                                            scalar1=bk_t[:, ft:ft + 1])

        # Normalized attention tiles go through a PE transpose whose input
        # comes from a short DVE chain; emitting the transpose immediately
        # would stall the in-order PE stream on DVE. Instead stage-1 (DVE
        # recip+scale) is emitted with the AV matmuls and the transposes are
        # deferred into the NEXT head's PE stream.
        pending_t = []
        _tcnt = [0]

        def _norm1(psav, attnT, po, fp, q0, bv_t, nm):
            rec = glob.tile([128, 1], F32, tag="rec", bufs=10, name=f"r{nm}")
            nc.vector.reciprocal(rec, psav[:, DK:DK + 1])
            an = glob.tile([128, DK], BF16, tag="an", bufs=10, name=f"n{nm}")
            nc.vector.tensor_scalar_mul(an, psav[:, 0:DK], rec)
            pending_t.append((an, attnT, po, fp, q0, bv_t))

        def flush_t():
            for an, attnT, po, fp, q0, bv_t in pending_t:
                _tcnt[0] += 1
                pst = ps_tile("t" if _tcnt[0] % 2 == 0 else "misc", 1,
                              shape=(DK, 128), dt=BF16, name=f"pt{_tcnt[0]}")
                nc.tensor.transpose(pst, an, ident)
                nc.vector.tensor_scalar_add(
                    out=attnT[po:po + DK, fp, q0:q0 + 128], in0=pst,
                    scalar1=bv_t[po:po + DK, fp:fp + 1])
            pending_t.clear()

        def attn_chunk(QT, KT, V_list, attnT, bv_t, cn, qoff, blocks, bias2,
                       diag_blk):
            for h in range(H):
                fp, po = h // 2, (h % 2) * DK
                ats = {}
                for blk in blocks:
                    ps = ps_tile("st", 2, name=f"pss_{h}_{cn}_{blk}")
                    for half in range(2):
                        st = blk * 2 + half
                        nc.tensor.matmul(
                            ps[:, half * 256:(half + 1) * 256],
                            lhsT=KT[po:po + DK, fp, st * 128:(st + 1) * 128],
                            rhs=QT[po:po + DK, fp, qoff:qoff + CHUNK],
                            start=True, stop=True)
                    at = glob.tile([128, 512], BF16, tag="at", bufs=16,
                                   name=f"a_{h}_{cn}_{blk}")
                    nc.scalar.activation(out=at, in_=ps, func=AF.Exp,
                                         scale=0.125,
                                         bias=bias2[:, blk:blk + 1])
                    if blk == diag_blk:
                        nc.vector.tensor_mul(at, at, dmask_sb)
                    ats[blk] = at
                flush_t()
                for qt in range(2):
                    psav = ps_tile("av" if qt % 2 == 0 else "big", 2,
                                   shape=(128, DK + 1),
                                   name=f"pav_{h}_{cn}_{qt}")
                    units = [(blk, half) for blk in blocks
                             for half in range(2)]
                    for i, (blk, half) in enumerate(units):
                        st = blk * 2 + half
                        nc.tensor.matmul(
                            psav,
                            lhsT=ats[blk][:, half * 256 + qt * 128:
                                          half * 256 + (qt + 1) * 128],
                            rhs=V_list[st][:, h, :],
                            start=(i == 0), stop=(i == len(units) - 1))
                    _norm1(psav, attnT, po, fp, qoff + qt * 128, bv_t,
                           f"s_{h}_{cn}_{qt}")
            flush_t()

        def attn_cross(QT, KT, V_list, attnT, bv_t):
            # two half-passes over s so only 8 exp tiles are live per head
            for h in range(H):
                fp, po = h // 2, (h % 2) * DK
                psavs = [ps_tile("av" if qt % 2 == 0 else "big", 2,
                                 shape=(128, DK + 1), name=f"pavc_{h}_{qt}")
                         for qt in range(4)]
                for half in range(2):
                    ats = {}
                    for st in range(half * 8, half * 8 + 8):
                        ps = ps_tile("st", 2, name=f"psc_{h}_{st}")
                        nc.tensor.matmul(
                            ps,
                            lhsT=KT[po:po + DK, fp, st * 128:(st + 1) * 128],
                            rhs=QT[po:po + DK, fp, :], start=True, stop=True)
                        at = glob.tile([128, 512], BF16, tag="at", bufs=16,
                                       name=f"ac_{h}_{st}")
                        nc.scalar.activation(out=at, in_=ps, func=AF.Exp,
                                             scale=0.125)
                        ats[st] = at
                    if half == 0:
                        flush_t()
                    for qt in range(4):
                        for st in range(half * 8, half * 8 + 8):
                            nc.tensor.matmul(
                                psavs[qt],
                                lhsT=ats[st][:, qt * 128:(qt + 1) * 128],
                                rhs=V_list[st][:, h, :],
                                start=(st == 0), stop=(st == 15))
                for qt in range(4):
                    _norm1(psavs[qt], attnT, po, fp, qt * 128, bv_t,
                           f"c_{h}_{qt}")
            flush_t()

        def wo_resid(attnT, WO_sb, bo_t, x_prev, x_out):
            for fo in range(FT):
                ps = ps_tile("big" if fo % 2 == 0 else "st", 2,
                             name=f"pwo_{fo}")
                for fi in range(FT):
                    nc.tensor.matmul(ps,
                                     lhsT=WO_sb[:, fi, fo * 128:(fo + 1) * 128],
                                     rhs=attnT[:, fi, :],
                                     start=(fi == 0), stop=(fi == FT - 1))
                nc.vector.scalar_tensor_tensor(
                    out=x_out[:, fo, :], in0=ps, scalar=bo_t[:, fo:fo + 1],
                    in1=x_prev[:, fo, :], op0=ALU.add, op1=ALU.add)

        def ln_stats(x_in, lbl):
            """-> (ps_mu, ps_rstd) broadcast PSUM tiles (tags misc/t)."""
            ps_sum = ps_tile("st", 2, shape=(1, TQ), name=f"psum_{lbl}")
            ps_sq = ps_tile("big", 2, shape=(1, TQ), name=f"psq_{lbl}")
            for fc in range(FT):
                xb = resid.tile([128, TQ], BF16, tag="sqb", bufs=3,
                                name=f"xb_{lbl}_{fc}")
                nc.vector.tensor_copy(out=xb, in_=x_in[:, fc, :])
                nc.tensor.matmul(ps_sum, lhsT=ones_b, rhs=xb,
                                 start=(fc == 0), stop=(fc == FT - 1))
                sqb = resid.tile([128, TQ], BF16, tag="sqb", bufs=3,
                                 name=f"sq_{lbl}_{fc}")
                nc.vector.tensor_mul(sqb, xb, xb)
                nc.tensor.matmul(ps_sq, lhsT=ones_b, rhs=sqb,
                                 start=(fc == 0), stop=(fc == FT - 1))
            mu = resid.tile([1, TQ], F32, tag="stat", bufs=2, name=f"mu_{lbl}")
            nc.scalar.activation(out=mu, in_=ps_sum, func=AF.Copy, scale=1.0 / D)
            msq = resid.tile([1, TQ], F32, tag="stat", bufs=2,
                             name=f"msq_{lbl}")
            nc.scalar.activation(out=msq, in_=ps_sq, func=AF.Copy, scale=1.0 / D)
            mu2 = resid.tile([128, TQ], F32, tag="sq", bufs=2,
                             name=f"mu2_{lbl}")
            nc.vector.tensor_mul(mu2[0:1, :], mu, mu)
            nc.vector.tensor_sub(msq, msq, mu2[0:1, :])  # msq <- var
            nc.scalar.activation(out=msq, in_=msq, func=AF.Sqrt, bias=eps_t,
                                 scale=1.0)              # msq <- std
            ps_mu = ps_tile("misc", 1, name=f"pmu_{lbl}")
            nc.tensor.matmul(ps_mu, lhsT=ones_row, rhs=mu, start=True,
                             stop=True)
            rstd = resid.tile([1, TQ], F32, tag="stat", bufs=2,
                              name=f"rstd_{lbl}")
            nc.vector.reciprocal(rstd, msq)
            ps_rstd = ps_tile("t", 1, name=f"prstd_{lbl}")
            nc.tensor.matmul(ps_rstd, lhsT=ones_row, rhs=rstd, start=True,
                             stop=True)
            return ps_mu, ps_rstd

        def ln_apply(stats, x_in, out_t, g_t, be_t, lbl, dma_out=None,
                     bf16_out=None):
            ps_mu, ps_rstd = stats
            for fc in range(FT):
                tmp = resid.tile([128, TQ], F32, tag="sq", bufs=2,
                                 name=f"t_{lbl}_{fc}")
                nc.vector.tensor_sub(tmp, x_in[:, fc, :], ps_mu)
                nc.vector.tensor_mul(tmp, tmp, ps_rstd)
                nc.vector.tensor_scalar(out=out_t[:, fc, :], in0=tmp,
                                        scalar1=g_t[:, fc:fc + 1],
                                        scalar2=be_t[:, fc:fc + 1],
                                        op0=ALU.mult, op1=ALU.add)
                if bf16_out is not None:
                    nc.vector.tensor_copy(out=bf16_out[:, fc, :],
                                          in_=out_t[:, fc, :])
                if dma_out is not None:
                    nc.sync.dma_start(out=dma_out[:, fc, :],
                                      in_=out_t[:, fc, :])

        # =========== program ===========
        QT = glob.tile([128, FT, TQ], BF16, tag="qt", bufs=1, name="QT_s")
        KT = glob.tile([128, FT, KV], BF16, tag="kt", bufs=1, name="KT_s")
        attnT = glob.tile([128, FT, TQ], BF16, tag="attnT", bufs=1,
                          name="attnT_s")
        V_s = []
        x_res = resid.tile([128, FT, TQ], F32, tag="res", bufs=2)
        x1p = resid.tile([128, FT, TQ], F32, tag="res", bufs=2, name="x1p")
        with ExitStack() as S1:
            wp = S1.enter_context(tc.tile_pool(name="wself", bufs=1))
            xq_b = wp.tile([128, FT, TQ], BF16, tag="xq", bufs=1)
            wq = glob.tile([128, FT, D], BF16, tag="wstream", bufs=2,
                           name="wq_s")
            for dc in range(FT):   # per-chunk loads so compute starts early
                nc.sync.dma_start(out=xq_b[:, dc, :], in_=xq_d[:, dc, :])
                nc.sync.dma_start(out=wq[:, dc, :], in_=w_d["wq_s"][:, dc, :])
            # small consts: one packed DMA on the gpsimd queue
            smallf = const.tile([128, 15 * FT + HT + 16], F32, name="c_small")
            nc.gpsimd.dma_start(out=smallf, in_=smallf_d)
            dmask_sb = const.tile([128, 512], BF16, name="c_dm")
            nc.gpsimd.dma_start(out=dmask_sb, in_=dmask_d)
            b_sb = {nm: smallf[:, i * FT:(i + 1) * FT]
                    for i, nm in enumerate(BIAS_NAMES)}
            b1_sb = smallf[:, 15 * FT:15 * FT + HT]
            biasa_sb = smallf[:, 15 * FT + HT:15 * FT + HT + 8]
            biasb_sb = smallf[:, 15 * FT + HT + 8:15 * FT + HT + 16]

            xp0 = wp.tile([128, FT, 256], BF16, tag="xkvp", bufs=2,
                          name="xkv_0")
            nc.sync.dma_start(out=xp0, in_=xkv_d[0])
            proj_q(QT, wq, xq_b, b_sb["bq_s"], "s")
            wv = wtile("wv_s")
            wk = wtile("wk_s")
            proj_kv_seg(KT, V_s, 0, xp0, wk, wv, b_sb["bk_s"], "v")
            nc.sync.dma_start(out=x_res, in_=xres_d)
            for seg in range(1, 5):
                xp = wp.tile([128, FT, 256], BF16, tag="xkvp", bufs=2,
                             name=f"xkv_{seg}")
                nc.sync.dma_start(out=xp, in_=xkv_d[seg])
                proj_kv_seg(KT, V_s, seg, xp, wk, wv, b_sb["bk_s"], "v")
            # chunk-A attention only needs kv tiles 0..9 (segs 0..4)
            attn_chunk(QT, KT, V_s, attnT, b_sb["bv_s"], "A", 0, BLOCKS_A,
                       biasa_sb, 0)
            for seg in range(5, NSEG):
                xp = wp.tile([128, FT, 256], BF16, tag="xkvp", bufs=2,
                             name=f"xkv_{seg}")
                nc.sync.dma_start(out=xp, in_=xkv_d[seg])
                proj_kv_seg(KT, V_s, seg, xp, wk, wv, b_sb["bk_s"], "v")

        attn_chunk(QT, KT, V_s, attnT, b_sb["bv_s"], "B", CHUNK, BLOCKS_B,
                   biasb_sb, 1)
        wo = wtile("wo_s")
        wo_resid(attnT, wo, b_sb["bo_s"], x_res, x1p)

        # LN1 stats now; the whole cross K/V projection runs while the
        # mean/rstd chain resolves; LN1 apply afterwards.
        st1 = ln_stats(x1p, "ln1")
        KT_c = glob.tile([128, FT, KV], BF16, tag="kt", bufs=1, name="KT_c")
        V_c = []
        wvc = wtile("wv_c")
        wkc = wtile("wk_c")
        for seg in range(NSEG):
            ep = glob.tile([128, FT, 256], BF16, tag="encp", bufs=2,
                           name=f"enc_{seg}")
            nc.sync.dma_start(out=ep, in_=enc_d[seg])
            proj_kv_seg(KT_c, V_c, seg, ep, wkc, wvc, b_sb["bk_c"], "vc")
        x1f = resid.tile([128, FT, TQ], F32, tag="res", bufs=2, name="x1f")
        x1n = resid.tile([128, FT, TQ], BF16, tag="xn", bufs=1, name="x1n")
        ln_apply(st1, x1p, x1f, b_sb["g1"], b_sb["be1"], "ln1", bf16_out=x1n)
        QT_c = glob.tile([128, FT, TQ], BF16, tag="qt", bufs=1, name="QT_c")
        wqc = wtile("wq_c")
        proj_q(QT_c, wqc, x1n, b_sb["bq_c"], "c")

        attnT_c = glob.tile([128, FT, TQ], BF16, tag="attnT", bufs=1,
                            name="attnT_c")
        x2p = resid.tile([128, FT, TQ], F32, tag="res", bufs=2, name="x2p")
        attn_cross(QT_c, KT_c, V_c, attnT_c, b_sb["bv_c"])
        woc = wtile("wo_c")
        wo_resid(attnT_c, woc, b_sb["bo_c"], x1f, x2p)
        st2 = ln_stats(x2p, "ln2")
        x2f = resid.tile([128, FT, TQ], F32, tag="res", bufs=2, name="x2f")
        x2n = resid.tile([128, FT, TQ], BF16, tag="xn", bufs=1, name="x2n")
        ln_apply(st2, x2p, x2f, b_sb["g2"], b_sb["be2"], "ln2", bf16_out=x2n)

        glob_ctx.close()

        # ---- FFN + LN3 + output ----
        x3 = resid.tile([128, FT, TQ], F32, tag="res", bufs=2, name="x3")
        out_sb = resid.tile([128, FT, TQ], F32, tag="res", bufs=2,
                            name="out_sb")
        with ExitStack() as S5:
            fp5 = S5.enter_context(tc.tile_pool(name="ffn", bufs=1))
            h_sb = fp5.tile([128, HT, TQ], BF16, tag="h", bufs=1, name="h_sb")
            # stream W1 in pieces (small first pieces so the first matmul
            # starts as early as possible after SBUF frees up)
            pieces = [2, 2, 4, 8, 8, 8]          # f-tiles per piece
            ht = 0
            for g, npc in enumerate(pieces):
                w1p = fp5.tile([128, FT, npc * 128], BF16, tag="w1", bufs=2,
                               padded_shape=[128, FT, 1024], name=f"w1_{g}")
                nc.sync.dma_start(
                    out=w1p, in_=w1_d[:, :, ht * 128:(ht + npc) * 128])
                for i in range(npc):
                    ps = ps_tile("big" if ht % 2 == 0 else "st", 2,
                                 name=f"pf1_{ht}")
                    for dc in range(FT):
                        nc.tensor.matmul(
                            ps, lhsT=w1p[:, dc, i * 128:(i + 1) * 128],
                            rhs=x2n[:, dc, :],
                            start=(dc == 0), stop=(dc == FT - 1))
                    # bias-add + relu + bf16 cast in one DVE op
                    nc.vector.tensor_scalar(out=h_sb[:, ht, :], in0=ps,
                                            scalar1=b1_sb[:, ht:ht + 1],
                                            scalar2=0.0,
                                            op0=ALU.add, op1=ALU.max)
                    ht += 1
            # W2: ht-outer with 8 concurrent PSUM accumulators (all banks),
            # streaming W2 in 4 pieces.
            tag8 = ["big", "big", "st", "st", "av", "av", "t", "misc"]
            ps8 = [ps_tile(tag8[fo], 2 if fo < 6 else 1, name=f"pf2_{fo}")
                   for fo in range(FT)]
            for g in range(4):
                w2p = fp5.tile([128, FT, D], BF16, tag="w2p", bufs=2,
                               name=f"w2_{g}")
                nc.sync.dma_start(out=w2p, in_=w2_d[:, g * 8:(g + 1) * 8, :])
                for i in range(8):
                    ht = g * 8 + i
                    for fo in range(FT):
                        nc.tensor.matmul(
                            ps8[fo], lhsT=w2p[:, i, fo * 128:(fo + 1) * 128],
                            rhs=h_sb[:, ht, :],
                            start=(ht == 0), stop=(ht == HT - 1))
            for fo in range(FT):
                nc.vector.scalar_tensor_tensor(
                    out=x3[:, fo, :], in0=ps8[fo],
                    scalar=b_sb["b2"][:, fo:fo + 1],
                    in1=x2f[:, fo, :], op0=ALU.add, op1=ALU.add)
            st3 = ln_stats(x3, "ln3")
            ln_apply(st3, x3, out_sb, b_sb["g3"], b_sb["be3"], "ln3",
                     dma_out=out_d)

    nc.compile()
    return nc


def _to_tiles(a2d, dt=ml_dtypes.bfloat16):
    """[P*128, F] -> [128, P, F] (SBUF tile layout), casting to dt."""
    p8, f = a2d.shape
    return np.ascontiguousarray(
        a2d.reshape(p8 // 128, 128, f).transpose(1, 0, 2).astype(dt))


def _seg_tiles(a2d):
    """[1024, NSEG*256] -> [NSEG, 128, 8, 256] bf16 (seg-major tiles)."""
    segs = [_to_tiles(a2d[:, s * 256:(s + 1) * 256]) for s in range(NSEG)]
    return np.ascontiguousarray(np.stack(segs))


def _vec_tiles(v, dt=np.float32):
    """[n*128] -> [128, n]"""
    return np.ascontiguousarray(v.reshape(-1, 128).T.astype(dt))


def _prep_core(c, dec, enc, consts):
    j = c % 4
    b = c // 4
    ja, jb = j, 7 - j
    rest = [ch for ch in range(0, jb) if ch != ja]
    qtok = np.r_[ja * CHUNK:(ja + 1) * CHUNK, jb * CHUNK:(jb + 1) * CHUNK]
    kvtok = np.concatenate(
        [qtok] + [np.arange(ch * CHUNK, (ch + 1) * CHUNK) for ch in rest])
    xq = dec[b][qtok]                       # [512, D]
    xkv = np.zeros((KV, D), np.float32)
    xkv[: len(kvtok)] = dec[b][kvtok]
    real_blocks = len(kvtok) // CHUNK

    # per-256-block additive exp biases (0 = attend, NEG = masked)
    biasa = np.full(8, NEG, np.float32)
    biasa[0] = 0.0                          # own diagonal block
    biasa[2:2 + ja] = 0.0                   # prior chunks in the window
    biasb = np.full(8, NEG, np.float32)
    biasb[:real_blocks] = 0.0

    m = dict(consts)
    m["xq"] = _to_tiles(xq.T)
    m["xres"] = _to_tiles(xq.T, np.float32)
    m["xkv"] = _seg_tiles(xkv.T)
    m["enc"] = _seg_tiles(enc[b].T)
    m["smallf"] = np.ascontiguousarray(np.concatenate(
        [m.pop("smallf_base"),
         np.repeat(biasa[None, :], 128, axis=0),
         np.repeat(biasb[None, :], 128, axis=0)], axis=1, dtype=np.float32))
    return m, (b, qtok)


def _prep_consts(inputs):
    bf = ml_dtypes.bfloat16
    c = {}
    for src, dst in (("Wq_s", "wq_s"), ("Wk_s", "wk_s"), ("Wv_s", "wv_s"),
                     ("Wq_c", "wq_c"), ("Wk_c", "wk_c"), ("Wv_c", "wv_c")):
        w = np.asarray(inputs[src], np.float32)           # [H, D, DK]
        c[dst] = _to_tiles(w.transpose(1, 0, 2).reshape(D, D))
    c["wo_s"] = _to_tiles(np.asarray(inputs["Wo_s"], np.float32))
    c["wo_c"] = _to_tiles(np.asarray(inputs["Wo_c"], np.float32))
    c["w1"] = _to_tiles(np.asarray(inputs["W1"], np.float32))
    c["w2"] = _to_tiles(np.asarray(inputs["W2"], np.float32))
    smalls = [_vec_tiles(np.asarray(inputs[nm], np.float32).reshape(-1))
              for nm in ("bq_s", "bk_s", "bv_s", "bo_s", "bq_c", "bk_c",
                         "bv_c", "bo_c", "b2", "g1", "be1", "g2", "be2",
                         "g3", "be3")]
    smalls.append(_vec_tiles(np.asarray(inputs["b1"], np.float32)))
    c["smallf_base"] = np.concatenate(smalls, axis=1)
    # causal diag mask M[s, q] = 1 if s <= q, packed [128, 512]
    M = (np.arange(CHUNK)[:, None] <= np.arange(CHUNK)[None, :]).astype(bf)
    c["dmask"] = np.ascontiguousarray(
        np.concatenate([M[0:128], M[128:256]], axis=1))
    return c


def _make_runner(nc):
    """Build the shard_map-jitted executable ONCE (run_bass_kernel_spmd
    re-traces and re-lowers per call, which costs seconds of host time)."""
    import jax
    import concourse.mybir as mybir_
    from concourse import bass2jax
    from jax.experimental.shard_map import shard_map
    from jax.sharding import Mesh, PartitionSpec

    bass2jax.install_neuronx_cc_hook()
    part_name = (nc.partition_id_tensor.name if nc.partition_id_tensor
                 else None)
    in_names, out_names, out_avals, zero_outs = [], [], [], []
    for alloc in nc.m.functions[0].allocations:
        if not isinstance(alloc, mybir_.MemoryLocationSet):
            continue
        name = alloc.memorylocations[0].name
        if alloc.kind == "ExternalInput":
            if name != part_name:
                in_names.append(name)
        elif alloc.kind == "ExternalOutput":
            shape = tuple(alloc.tensor_shape)
            dtype = mybir_.dt.np(alloc.dtype)
            out_names.append(name)
            out_avals.append(jax.core.ShapedArray(shape, dtype))
            zero_outs.append(np.zeros(shape, dtype))
    n_params = len(in_names)
    all_names = in_names + out_names
    if part_name is not None:
        all_names = all_names + [part_name]
    donate = tuple(range(n_params, n_params + len(out_names)))

    def _body(*args):
        operands = list(args)
        if part_name is not None:
            operands.append(bass2jax.partition_id_tensor())
        outs = bass2jax._bass_exec_p.bind(
            *operands, out_avals=tuple(out_avals), in_names=tuple(all_names),
            out_names=tuple(out_names), lowering_input_output_aliases=(),
            sim_require_finite=True, sim_require_nnan=True, nc=nc)
        return tuple(outs)

    # inputs identical on every core are passed replicated (uploaded once)
    REPL = {"wq_s", "wk_s", "wv_s", "wo_s", "wq_c", "wk_c", "wv_c", "wo_c",
            "w1", "w2", "dmask"}
    in_specs = tuple(PartitionSpec() if nm in REPL else PartitionSpec("core")
                     for nm in in_names) + \
        (PartitionSpec("core"),) * len(out_names)
    devices = jax.devices()[:N_CORES]
    mesh = Mesh(np.asarray(devices), ("core",))
    sharded = jax.jit(
        shard_map(_body, mesh=mesh, in_specs=in_specs,
                  out_specs=(PartitionSpec("core"),) * len(out_names),
                  check_rep=False),
        donate_argnums=donate, keep_unused=True)

    def run(in_maps):
        concat_in = [
            in_maps[0][nm] if nm in REPL else
            np.concatenate([in_maps[c][nm] for c in range(N_CORES)], axis=0)
            for nm in in_names]
        concat_zero = [
            np.zeros((N_CORES * z.shape[0], *z.shape[1:]), z.dtype)
            for z in zero_outs]
        out_arrs = sharded(*concat_in, *concat_zero)
        return [
            {nm: np.asarray(out_arrs[i]).reshape(N_CORES, *out_avals[i].shape)[c]
             for i, nm in enumerate(out_names)}
            for c in range(N_CORES)]

    return run


def kernel(**inputs):
    global _BUILT
    if _BUILT is None:
        nc = _build()
        try:
            from concourse._compat import axon_active
            under_axon = axon_active()
        except ImportError:
            under_axon = False
        if under_axon:
            _BUILT = _make_runner(nc)
        else:
            def _native_run(in_maps, _nc=nc):
                res = run_bass_kernel_spmd(_nc, in_maps,
                                           core_ids=list(range(N_CORES)))
                return res.results
            _BUILT = _native_run
    run = _BUILT

    dec = np.asarray(inputs["dec_input"], np.float32)
    enc = np.asarray(inputs["enc_output"], np.float32)
    consts = _prep_consts(inputs)
    in_maps = []
    metas = []
    for cix in range(N_CORES):
        m, meta = _prep_core(cix, dec, enc, consts)
        in_maps.append(m)
        metas.append(meta)

    results = run(in_maps)

    out = np.empty((B, T, D), np.float32)
    for cix in range(N_CORES):
        b, qtok = metas[cix]
        tiles = results[cix]["out"]           # [128, FT, TQ]
        core_t = tiles.transpose(1, 0, 2).reshape(D, TQ)
        out[b, qtok, :] = core_t.T
    return out


# revision 52
# speedup vs baseline: 1.4315x; 1.2048x over previous
"""Transformer decoder layer (self-attn + cross-attn + FFN, post-LN) on 8
Trainium2 NeuronCores, sequence-parallel with zero collectives.

Sharding: core c -> batch b = c//4, causal-balanced chunk pair (j, 7-j) of
256 tokens each (j = c%4), so every core owns 512 query tokens with equal
total causal attention area. Weights are replicated; K/V projections are
recomputed per core. All per-core differences are expressed through input
DATA (token reordering + additive exp-bias masks), so a single SPMD program
serves all 8 cores.

Layout: activations are kept feature-major [d_partition, token_free] so
every matmul contracts along SBUF partitions with weights as the stationary
operand. Scores are computed transposed (S^T = [s, q]) which makes softmax
need no cross-partition reduction: exp on ScalarE (scores are O(0.5), so no
max subtraction), denominator via an extra ones-column appended to V.
Matmuls run in bf16 with fp32 PSUM accumulation; the residual stream and
layernorm arithmetic stay fp32 (stat sums in bf16, mean/rstd broadcasts via
fp32 matmuls).

Overlap structure: chunk-A attention (which only needs the first 1280 kv
tokens) is emitted mid-way through the K/V projection segments; the whole
cross K/V projection is sandwiched between LN1's stats and apply so the LN
latency chain hides under projection matmuls.
"""

import sys

if "/opt/trn_rl_repo" not in sys.path:
    sys.path.insert(0, "/opt/trn_rl_repo")

from contextlib import ExitStack

import numpy as np
import ml_dtypes

import concourse.bass as bass
import concourse.bacc as bacc
import concourse.tile as tile
import concourse.mybir as mybir
from concourse.bass_utils import run_bass_kernel_spmd
from concourse.masks import make_identity

F32 = mybir.dt.float32
BF16 = mybir.dt.bfloat16
AF = mybir.ActivationFunctionType
ALU = mybir.AluOpType

D = 1024
H = 16
DK = 64
DFF = 4096
B = 2
T = 2048
N_CORES = 8
CHUNK = 256
TQ = 512          # query tokens per core
KV = 2048         # padded kv layout length (self), enc length (cross)
FT = D // 128     # 8 f-tiles
HT = DFF // 128   # 32 ffn tiles
NSEG = 8          # kv/enc DMA-streaming segments of 256 tokens
NEG = -50.0       # additive pre-exp mask (exp(-50) ~ 2e-22)

# self-attn 256-token s-block schedules over the kv layout
# [A(256) | B(256) | rest... | pad]  (block = 2 s-tiles of 128):
BLOCKS_A = [0, 2, 3, 4]    # own diag + 768-token prior window
BLOCKS_B = list(range(8))  # everything (pads masked via bias)

_BUILT = None
_NC = None


def _build():
    nc = bacc.Bacc("TRN2", target_bir_lowering=False, debug=False,
                   num_devices=N_CORES)

    def din(name, shape, dt):
        return nc.dram_tensor(name, shape, dt, kind="ExternalInput").ap()

    xq_d = din("xq", [128, FT, TQ], BF16)
    xres_d = din("xres", [128, FT, TQ], F32)
    xkv_d = din("xkv", [NSEG, 128, FT, 256], BF16)    # seg-major
    enc_d = din("enc", [NSEG, 128, FT, 256], BF16)    # seg-major
    w_d = {}
    for nm in ("wq_s", "wk_s", "wv_s", "wo_s", "wq_c", "wk_c", "wv_c", "wo_c"):
        w_d[nm] = din(nm, [128, FT, D], BF16)
    w1_d = din("w1", [128, FT, DFF], BF16)
    w2_d = din("w2", [128, HT, D], BF16)
    # all small fp32 vectors packed into one tensor: 15 biases/ln params of
    # [128, 8], then b1 [128, 32], biasa2 [128, 8], biasb2 [128, 8]
    BIAS_NAMES = ("bq_s", "bk_s", "bv_s", "bo_s", "bq_c", "bk_c", "bv_c",
                  "bo_c", "b2", "g1", "be1", "g2", "be2", "g3", "be3")
    smallf_d = din("smallf", [128, 15 * FT + HT + 16], F32)
    dmask_d = din("dmask", [128, 512], BF16)
    out_d = nc.dram_tensor("out", [128, FT, TQ], F32, kind="ExternalOutput").ap()

    with tile.TileContext(nc) as tc, ExitStack() as S:
        const = S.enter_context(tc.tile_pool(name="const", bufs=1))
        pp = S.enter_context(tc.tile_pool(name="ps", bufs=1, space="PSUM"))
        resid = S.enter_context(tc.tile_pool(name="resid", bufs=1))

        ident = const.tile([128, 128], BF16)
        make_identity(nc, ident)
        ones_b = const.tile([128, 1], BF16)
        nc.vector.memset(ones_b, 1.0)
        ones_row = const.tile([1, 128], F32)
        nc.vector.memset(ones_row, 1.0)
        eps_t = const.tile([1, 1], F32)
        nc.vector.memset(eps_t, 1e-5)

        glob_ctx = ExitStack()
        glob = glob_ctx.enter_context(tc.tile_pool(name="glob", bufs=1))

        # =========== helpers ===========
        def ps_tile(tag, bufs, shape=(128, 512), dt=F32, name="ps"):
            return pp.tile(list(shape), dt, tag=tag, bufs=bufs, name=name)

        def wtile(nm):
            t = glob.tile([128, FT, D], BF16, tag="wstream", bufs=2, name=nm)
            for dc in range(FT):     # per-chunk so first consumers start early
                nc.sync.dma_start(out=t[:, dc, :], in_=w_d[nm][:, dc, :])
            return t

        TAG8 = ["big", "big", "st", "st", "av", "av", "t", "misc"]
        BUF8 = [2, 2, 2, 2, 2, 2, 1, 1]

        def proj_q(out_t, W_sb, X_sb, bias_t, lbl):
            # dc-outer with 8 concurrent accumulators: the first matmul only
            # needs the first d-chunk of W and X (fast start after DMA).
            ps8 = [ps_tile(TAG8[ft], BUF8[ft], name=f"pjq_{lbl}_{ft}")
                   for ft in range(FT)]
            for dc in range(FT):
                for ft in range(FT):
                    nc.tensor.matmul(
                        ps8[ft], lhsT=W_sb[:, dc, ft * 128:(ft + 1) * 128],
                        rhs=X_sb[:, dc, :],
                        start=(dc == 0), stop=(dc == FT - 1))
            for ft in range(FT):
                nc.scalar.activation(out=out_t[:, ft, :], in_=ps8[ft],
                                     func=AF.Identity,
                                     bias=bias_t[:, ft:ft + 1], scale=1.0)

        def proj_kv_seg(KT, V_list, seg, X_piece, WK_sb, WV_sb, bk_t, vtag):
            """one 256-token segment of V (token-major) then K^T (f-major)."""
            sl = slice(seg * 256, (seg + 1) * 256)
            for sti in range(2):
                st = seg * 2 + sti
                vt = glob.tile([128, H, DK + 1], BF16, tag="v", bufs=16,
                               name=f"v_{vtag}_{st}")
                for half in range(2):
                    ps = ps_tile("big" if half == 0 else "st", 2,
                                 name=f"pv_{vtag}_{st}_{half}")
                    for dc in range(FT):
                        nc.tensor.matmul(
                            ps,
                            lhsT=X_piece[:, dc, sti * 128:(sti + 1) * 128],
                            rhs=WV_sb[:, dc, half * 512:(half + 1) * 512],
                            start=(dc == 0), stop=(dc == FT - 1))
                    nc.vector.tensor_copy(
                        out=vt[:, half * 8:(half + 1) * 8, 0:DK],
                        in_=ps.rearrange("p (a b) -> p a b", b=DK))
                nc.vector.memset(vt[:, :, DK:DK + 1], 1.0)
                V_list.append(vt)
            for ft in range(FT):
                ps = ps_tile("big" if ft % 2 == 0 else "st", 2,
                             shape=(128, 256), name=f"pjk_{vtag}_{seg}_{ft}")
                for dc in range(FT):
                    nc.tensor.matmul(
                        ps, lhsT=WK_sb[:, dc, ft * 128:(ft + 1) * 128],
                        rhs=X_piece[:, dc, :],
                        start=(dc == 0), stop=(dc == FT - 1))
                nc.vector.tensor_scalar_add(out=KT[:, ft, sl], in0=ps,
                                            scalar1=bk_t[:, ft:ft + 1])

        # Normalized attention tiles go through a PE transpose whose input
        # comes from a short DVE chain; emitting the transpose immediately
        # would stall the in-order PE stream on DVE. Instead stage-1 (DVE
        # recip+scale) is emitted with the AV matmuls and the transposes are
        # deferred into the NEXT head's PE stream.
        pending_t = []
        _tcnt = [0]

        def _norm1(psav, attnT, po, fp, q0, bv_t, nm):
            rec = glob.tile([128, 1], F32, tag="rec", bufs=10, name=f"r{nm}")
            nc.vector.reciprocal(rec, psav[:, DK:DK + 1])
            an = glob.tile([128, DK], BF16, tag="an", bufs=10, name=f"n{nm}")
            nc.vector.tensor_scalar_mul(an, psav[:, 0:DK], rec)
            pending_t.append((an, attnT, po, fp, q0, bv_t))

        def flush_t():
            for an, attnT, po, fp, q0, bv_t in pending_t:
                _tcnt[0] += 1
                pst = ps_tile("t" if _tcnt[0] % 2 == 0 else "misc", 1,
                              shape=(DK, 128), dt=BF16, name=f"pt{_tcnt[0]}")
                nc.tensor.transpose(pst, an, ident)
                nc.vector.tensor_scalar_add(
                    out=attnT[po:po + DK, fp, q0:q0 + 128], in0=pst,
                    scalar1=bv_t[po:po + DK, fp:fp + 1])
            pending_t.clear()

        def attn_chunk(QT, KT, V_list, attnT, bv_t, cn, qoff, blocks, bias2,
                       diag_blk):
            for h in range(H):
                fp, po = h // 2, (h % 2) * DK
                ats = {}
                for blk in blocks:
                    ps = ps_tile("st", 2, name=f"pss_{h}_{cn}_{blk}")
                    for half in range(2):
                        st = blk * 2 + half
                        nc.tensor.matmul(
                            ps[:, half * 256:(half + 1) * 256],
                            lhsT=KT[po:po + DK, fp, st * 128:(st + 1) * 128],
                            rhs=QT[po:po + DK, fp, qoff:qoff + CHUNK],
                            start=True, stop=True)
                    at = glob.tile([128, 512], BF16, tag="at", bufs=16,
                                   name=f"a_{h}_{cn}_{blk}")
                    nc.scalar.activation(out=at, in_=ps, func=AF.Exp,
                                         scale=0.125,
                                         bias=bias2[:, blk:blk + 1])
                    if blk == diag_blk:
                        nc.vector.tensor_mul(at, at, dmask_sb)
                    ats[blk] = at
                flush_t()
                for qt in range(2):
                    psav = ps_tile("av" if qt % 2 == 0 else "big", 2,
                                   shape=(128, DK + 1),
                                   name=f"pav_{h}_{cn}_{qt}")
                    units = [(blk, half) for blk in blocks
                             for half in range(2)]
                    for i, (blk, half) in enumerate(units):
                        st = blk * 2 + half
                        nc.tensor.matmul(
                            psav,
                            lhsT=ats[blk][:, half * 256 + qt * 128:
                                          half * 256 + (qt + 1) * 128],
                            rhs=V_list[st][:, h, :],
                            start=(i == 0), stop=(i == len(units) - 1))
                    _norm1(psav, attnT, po, fp, qoff + qt * 128, bv_t,
                           f"s_{h}_{cn}_{qt}")
            flush_t()

        def attn_cross(QT, KT, V_list, attnT, bv_t):
            # two half-passes over s so only 8 exp tiles are live per head
            for h in range(H):
                fp, po = h // 2, (h % 2) * DK
                psavs = [ps_tile("av" if qt % 2 == 0 else "big", 2,
                                 shape=(128, DK + 1), name=f"pavc_{h}_{qt}")
                         for qt in range(4)]
                for half in range(2):
                    ats = {}
                    for st in range(half * 8, half * 8 + 8):
                        ps = ps_tile("st", 2, name=f"psc_{h}_{st}")
                        nc.tensor.matmul(
                            ps,
                            lhsT=KT[po:po + DK, fp, st * 128:(st + 1) * 128],
                            rhs=QT[po:po + DK, fp, :], start=True, stop=True)
                        at = glob.tile([128, 512], BF16, tag="at", bufs=16,
                                       name=f"ac_{h}_{st}")
                        nc.scalar.activation(out=at, in_=ps, func=AF.Exp,
                                             scale=0.125)
                        ats[st] = at
                    if half == 0:
                        flush_t()
                    for qt in range(4):
                        for st in range(half * 8, half * 8 + 8):
                            nc.tensor.matmul(
                                psavs[qt],
                                lhsT=ats[st][:, qt * 128:(qt + 1) * 128],
                                rhs=V_list[st][:, h, :],
                                start=(st == 0), stop=(st == 15))
                for qt in range(4):
                    _norm1(psavs[qt], attnT, po, fp, qt * 128, bv_t,
                           f"c_{h}_{qt}")
            flush_t()

        def wo_resid(attnT, WO_sb, bo_t, x_prev, x_out):
            for fo in range(FT):
                ps = ps_tile("big" if fo % 2 == 0 else "st", 2,
                             name=f"pwo_{fo}")
                for fi in range(FT):
                    nc.tensor.matmul(ps,
                                     lhsT=WO_sb[:, fi, fo * 128:(fo + 1) * 128],
                                     rhs=attnT[:, fi, :],
                                     start=(fi == 0), stop=(fi == FT - 1))
                nc.vector.scalar_tensor_tensor(
                    out=x_out[:, fo, :], in0=ps, scalar=bo_t[:, fo:fo + 1],
                    in1=x_prev[:, fo, :], op0=ALU.add, op1=ALU.add)

        def ln_stats(x_in, lbl):
            """-> (ps_mu, ps_rstd) broadcast PSUM tiles (tags misc/t)."""
            ps_sum = ps_tile("st", 2, shape=(1, TQ), name=f"psum_{lbl}")
            ps_sq = ps_tile("big", 2, shape=(1, TQ), name=f"psq_{lbl}")
            for fc in range(FT):
                xb = resid.tile([128, TQ], BF16, tag="sqb", bufs=3,
                                name=f"xb_{lbl}_{fc}")
                nc.vector.tensor_copy(out=xb, in_=x_in[:, fc, :])
                nc.tensor.matmul(ps_sum, lhsT=ones_b, rhs=xb,
                                 start=(fc == 0), stop=(fc == FT - 1))
                sqb = resid.tile([128, TQ], BF16, tag="sqb", bufs=3,
                                 name=f"sq_{lbl}_{fc}")
                nc.vector.tensor_mul(sqb, xb, xb)
                nc.tensor.matmul(ps_sq, lhsT=ones_b, rhs=sqb,
                                 start=(fc == 0), stop=(fc == FT - 1))
            mu = resid.tile([1, TQ], F32, tag="stat", bufs=2, name=f"mu_{lbl}")
            nc.scalar.activation(out=mu, in_=ps_sum, func=AF.Copy, scale=1.0 / D)
            msq = resid.tile([1, TQ], F32, tag="stat", bufs=2,
                             name=f"msq_{lbl}")
            nc.scalar.activation(out=msq, in_=ps_sq, func=AF.Copy, scale=1.0 / D)
            mu2 = resid.tile([128, TQ], F32, tag="sq", bufs=2,
                             name=f"mu2_{lbl}")
            nc.vector.tensor_mul(mu2[0:1, :], mu, mu)
            nc.vector.tensor_sub(msq, msq, mu2[0:1, :])  # msq <- var
            nc.scalar.activation(out=msq, in_=msq, func=AF.Sqrt, bias=eps_t,
                                 scale=1.0)              # msq <- std
            ps_mu = ps_tile("misc", 1, name=f"pmu_{lbl}")
            nc.tensor.matmul(ps_mu, lhsT=ones_row, rhs=mu, start=True,
                             stop=True)
            rstd = resid.tile([1, TQ], F32, tag="stat", bufs=2,
                              name=f"rstd_{lbl}")
            nc.vector.reciprocal(rstd, msq)
            ps_rstd = ps_tile("t", 1, name=f"prstd_{lbl}")
            nc.tensor.matmul(ps_rstd, lhsT=ones_row, rhs=rstd, start=True,
                             stop=True)
            return ps_mu, ps_rstd

        def ln_apply(stats, x_in, out_t, g_t, be_t, lbl, dma_out=None,
                     bf16_out=None):
            ps_mu, ps_rstd = stats
            for fc in range(FT):
                tmp = resid.tile([128, TQ], F32, tag="sq", bufs=2,
                                 name=f"t_{lbl}_{fc}")
                nc.vector.tensor_sub(tmp, x_in[:, fc, :], ps_mu)
                nc.vector.tensor_mul(tmp, tmp, ps_rstd)
                nc.vector.tensor_scalar(out=out_t[:, fc, :], in0=tmp,
                                        scalar1=g_t[:, fc:fc + 1],
                                        scalar2=be_t[:, fc:fc + 1],
                                        op0=ALU.mult, op1=ALU.add)
                if bf16_out is not None:
                    nc.vector.tensor_copy(out=bf16_out[:, fc, :],
                                          in_=out_t[:, fc, :])
                if dma_out is not None:
                    nc.sync.dma_start(out=dma_out[:, fc, :],
                                      in_=out_t[:, fc, :])

        # =========== program ===========
        QT = glob.tile([128, FT, TQ], BF16, tag="qt", bufs=1, name="QT_s")
        KT = glob.tile([128, FT, KV], BF16, tag="kt", bufs=1, name="KT_s")
        attnT = glob.tile([128, FT, TQ], BF16, tag="attnT", bufs=1,
                          name="attnT_s")
        V_s = []
        x_res = resid.tile([128, FT, TQ], F32, tag="res", bufs=2)
        x1p = resid.tile([128, FT, TQ], F32, tag="res", bufs=2, name="x1p")
        with ExitStack() as S1:
            wp = S1.enter_context(tc.tile_pool(name="wself", bufs=1))
            xq_b = wp.tile([128, FT, TQ], BF16, tag="xq", bufs=1)
            wq = glob.tile([128, FT, D], BF16, tag="wstream", bufs=2,
                           name="wq_s")
            for dc in range(FT):   # per-chunk loads so compute starts early
                nc.sync.dma_start(out=xq_b[:, dc, :], in_=xq_d[:, dc, :])
                nc.sync.dma_start(out=wq[:, dc, :], in_=w_d["wq_s"][:, dc, :])
            # small consts: one packed DMA on the gpsimd queue
            smallf = const.tile([128, 15 * FT + HT + 16], F32, name="c_small")
            nc.gpsimd.dma_start(out=smallf, in_=smallf_d)
            dmask_sb = const.tile([128, 512], BF16, name="c_dm")
            nc.gpsimd.dma_start(out=dmask_sb, in_=dmask_d)
            b_sb = {nm: smallf[:, i * FT:(i + 1) * FT]
                    for i, nm in enumerate(BIAS_NAMES)}
            b1_sb = smallf[:, 15 * FT:15 * FT + HT]
            biasa_sb = smallf[:, 15 * FT + HT:15 * FT + HT + 8]
            biasb_sb = smallf[:, 15 * FT + HT + 8:15 * FT + HT + 16]

            xp0 = wp.tile([128, FT, 256], BF16, tag="xkvp", bufs=2,
                          name="xkv_0")
            nc.sync.dma_start(out=xp0, in_=xkv_d[0])
            proj_q(QT, wq, xq_b, b_sb["bq_s"], "s")
            wv = wtile("wv_s")
            wk = wtile("wk_s")
            proj_kv_seg(KT, V_s, 0, xp0, wk, wv, b_sb["bk_s"], "v")
            nc.sync.dma_start(out=x_res, in_=xres_d)
            for seg in range(1, 5):
                xp = wp.tile([128, FT, 256], BF16, tag="xkvp", bufs=2,
                             name=f"xkv_{seg}")
                nc.sync.dma_start(out=xp, in_=xkv_d[seg])
                proj_kv_seg(KT, V_s, seg, xp, wk, wv, b_sb["bk_s"], "v")
            # chunk-A attention only needs kv tiles 0..9 (segs 0..4)
            attn_chunk(QT, KT, V_s, attnT, b_sb["bv_s"], "A", 0, BLOCKS_A,
                       biasa_sb, 0)
            for seg in range(5, NSEG):
                xp = wp.tile([128, FT, 256], BF16, tag="xkvp", bufs=2,
                             name=f"xkv_{seg}")
                nc.sync.dma_start(out=xp, in_=xkv_d[seg])
                proj_kv_seg(KT, V_s, seg, xp, wk, wv, b_sb["bk_s"], "v")

        attn_chunk(QT, KT, V_s, attnT, b_sb["bv_s"], "B", CHUNK, BLOCKS_B,
                   biasb_sb, 1)
        wo = wtile("wo_s")
        wo_resid(attnT, wo, b_sb["bo_s"], x_res, x1p)

        # LN1 stats now; the whole cross K/V projection runs while the
        # mean/rstd chain resolves; LN1 apply afterwards.
        st1 = ln_stats(x1p, "ln1")
        KT_c = glob.tile([128, FT, KV], BF16, tag="kt", bufs=1, name="KT_c")
        V_c = []
        wvc = wtile("wv_c")
        wkc = wtile("wk_c")
        for seg in range(NSEG):
            ep = glob.tile([128, FT, 256], BF16, tag="encp", bufs=2,
                           name=f"enc_{seg}")
            nc.sync.dma_start(out=ep, in_=enc_d[seg])
            proj_kv_seg(KT_c, V_c, seg, ep, wkc, wvc, b_sb["bk_c"], "vc")
        x1f = resid.tile([128, FT, TQ], F32, tag="res", bufs=2, name="x1f")
        x1n = resid.tile([128, FT, TQ], BF16, tag="xn", bufs=1, name="x1n")
        ln_apply(st1, x1p, x1f, b_sb["g1"], b_sb["be1"], "ln1", bf16_out=x1n)
        QT_c = glob.tile([128, FT, TQ], BF16, tag="qt", bufs=1, name="QT_c")
        wqc = wtile("wq_c")
        proj_q(QT_c, wqc, x1n, b_sb["bq_c"], "c")

        attnT_c = glob.tile([128, FT, TQ], BF16, tag="attnT", bufs=1,
                            name="attnT_c")
        x2p = resid.tile([128, FT, TQ], F32, tag="res", bufs=2, name="x2p")
        attn_cross(QT_c, KT_c, V_c, attnT_c, b_sb["bv_c"])
        woc = wtile("wo_c")
        wo_resid(attnT_c, woc, b_sb["bo_c"], x1f, x2p)
        st2 = ln_stats(x2p, "ln2")
        x2f = resid.tile([128, FT, TQ], F32, tag="res", bufs=2, name="x2f")
        x2n = resid.tile([128, FT, TQ], BF16, tag="xn", bufs=1, name="x2n")
        ln_apply(st2, x2p, x2f, b_sb["g2"], b_sb["be2"], "ln2", bf16_out=x2n)

        glob_ctx.close()

        # ---- FFN + LN3 + output ----
        x3 = resid.tile([128, FT, TQ], F32, tag="res", bufs=2, name="x3")
        out_sb = resid.tile([128, FT, TQ], F32, tag="res", bufs=2,
                            name="out_sb")
        with ExitStack() as S5:
            fp5 = S5.enter_context(tc.tile_pool(name="ffn", bufs=1))
            h_sb = fp5.tile([128, HT, TQ], BF16, tag="h", bufs=1, name="h_sb")
            # stream W1 in pieces (small first pieces so the first matmul
            # starts as early as possible after SBUF frees up)
            pieces = [2, 2, 4, 8, 8, 8]          # f-tiles per piece
            ht = 0
            for g, npc in enumerate(pieces):
                w1p = fp5.tile([128, FT, npc * 128], BF16, tag="w1", bufs=2,
                               padded_shape=[128, FT, 1024], name=f"w1_{g}")
                nc.sync.dma_start(
                    out=w1p, in_=w1_d[:, :, ht * 128:(ht + npc) * 128])
                for i in range(npc):
                    ps = ps_tile("big" if ht % 2 == 0 else "st", 2,
                                 name=f"pf1_{ht}")
                    for dc in range(FT):
                        nc.tensor.matmul(
                            ps, lhsT=w1p[:, dc, i * 128:(i + 1) * 128],
                            rhs=x2n[:, dc, :],
                            start=(dc == 0), stop=(dc == FT - 1))
                    # bias-add + relu + bf16 cast in one DVE op
                    nc.vector.tensor_scalar(out=h_sb[:, ht, :], in0=ps,
                                            scalar1=b1_sb[:, ht:ht + 1],
                                            scalar2=0.0,
                                            op0=ALU.add, op1=ALU.max)
                    ht += 1
            # W2: ht-outer with 8 concurrent PSUM accumulators (all banks),
            # streaming W2 in 4 pieces.
            tag8 = ["big", "big", "st", "st", "av", "av", "t", "misc"]
            ps8 = [ps_tile(tag8[fo], 2 if fo < 6 else 1, name=f"pf2_{fo}")
                   for fo in range(FT)]
            for g in range(4):
                w2p = fp5.tile([128, FT, D], BF16, tag="w2p", bufs=2,
                               name=f"w2_{g}")
                nc.sync.dma_start(out=w2p, in_=w2_d[:, g * 8:(g + 1) * 8, :])
                for i in range(8):
                    ht = g * 8 + i
                    for fo in range(FT):
                        nc.tensor.matmul(
                            ps8[fo], lhsT=w2p[:, i, fo * 128:(fo + 1) * 128],
                            rhs=h_sb[:, ht, :],
                            start=(ht == 0), stop=(ht == HT - 1))
            for fo in range(FT):
                nc.vector.scalar_tensor_tensor(
                    out=x3[:, fo, :], in0=ps8[fo],
                    scalar=b_sb["b2"][:, fo:fo + 1],
                    in1=x2f[:, fo, :], op0=ALU.add, op1=ALU.add)
            st3 = ln_stats(x3, "ln3")
            ln_apply(st3, x3, out_sb, b_sb["g3"], b_sb["be3"], "ln3",
                     dma_out=out_d)

    nc.compile()
    return nc


def _to_tiles(a2d, dt=ml_dtypes.bfloat16):
    """[P*128, F] -> [128, P, F] (SBUF tile layout), casting to dt."""
    p8, f = a2d.shape
    return np.ascontiguousarray(
        a2d.reshape(p8 // 128, 128, f).transpose(1, 0, 2).astype(dt))


def _seg_tiles(a2d):
    """[1024, NSEG*256] -> [NSEG, 128, 8, 256] bf16 (seg-major tiles)."""
    segs = [_to_tiles(a2d[:, s * 256:(s + 1) * 256]) for s in range(NSEG)]
    return np.ascontiguousarray(np.stack(segs))


def _vec_tiles(v, dt=np.float32):
    """[n*128] -> [128, n]"""
    return np.ascontiguousarray(v.reshape(-1, 128).T.astype(dt))


def _prep_core(c, dec, enc, consts):
    j = c % 4
    b = c // 4
    ja, jb = j, 7 - j
    rest = [ch for ch in range(0, jb) if ch != ja]
    qtok = np.r_[ja * CHUNK:(ja + 1) * CHUNK, jb * CHUNK:(jb + 1) * CHUNK]
    kvtok = np.concatenate(
        [qtok] + [np.arange(ch * CHUNK, (ch + 1) * CHUNK) for ch in rest])
    xq = dec[b][qtok]                       # [512, D]
    xkv = np.zeros((KV, D), np.float32)
    xkv[: len(kvtok)] = dec[b][kvtok]
    real_blocks = len(kvtok) // CHUNK

    # per-256-block additive exp biases (0 = attend, NEG = masked)
    biasa = np.full(8, NEG, np.float32)
    biasa[0] = 0.0                          # own diagonal block
    biasa[2:2 + ja] = 0.0                   # prior chunks in the window
    biasb = np.full(8, NEG, np.float32)
    biasb[:real_blocks] = 0.0

    m = dict(consts)
    m["xq"] = _to_tiles(xq.T)
    m["xres"] = _to_tiles(xq.T, np.float32)
    m["xkv"] = _seg_tiles(xkv.T)
    m["enc"] = _seg_tiles(enc[b].T)
    m["smallf"] = np.ascontiguousarray(np.concatenate(
        [m.pop("smallf_base"),
         np.repeat(biasa[None, :], 128, axis=0),
         np.repeat(biasb[None, :], 128, axis=0)], axis=1, dtype=np.float32))
    return m, (b, qtok)


def _prep_consts(inputs):
    bf = ml_dtypes.bfloat16
    c = {}
    for src, dst in (("Wq_s", "wq_s"), ("Wk_s", "wk_s"), ("Wv_s", "wv_s"),
                     ("Wq_c", "wq_c"), ("Wk_c", "wk_c"), ("Wv_c", "wv_c")):
        w = np.asarray(inputs[src], np.float32)           # [H, D, DK]
        c[dst] = _to_tiles(w.transpose(1, 0, 2).reshape(D, D))
    c["wo_s"] = _to_tiles(np.asarray(inputs["Wo_s"], np.float32))
    c["wo_c"] = _to_tiles(np.asarray(inputs["Wo_c"], np.float32))
    c["w1"] = _to_tiles(np.asarray(inputs["W1"], np.float32))
    c["w2"] = _to_tiles(np.asarray(inputs["W2"], np.float32))
    smalls = [_vec_tiles(np.asarray(inputs[nm], np.float32).reshape(-1))
              for nm in ("bq_s", "bk_s", "bv_s", "bo_s", "bq_c", "bk_c",
                         "bv_c", "bo_c", "b2", "g1", "be1", "g2", "be2",
                         "g3", "be3")]
    smalls.append(_vec_tiles(np.asarray(inputs["b1"], np.float32)))
    c["smallf_base"] = np.concatenate(smalls, axis=1)
    # causal diag mask M[s, q] = 1 if s <= q, packed [128, 512]
    M = (np.arange(CHUNK)[:, None] <= np.arange(CHUNK)[None, :]).astype(bf)
    c["dmask"] = np.ascontiguousarray(
        np.concatenate([M[0:128], M[128:256]], axis=1))
    return c


def _make_runner(nc):
    """Build the shard_map-jitted executable ONCE (run_bass_kernel_spmd
    re-traces and re-lowers per call, which costs seconds of host time)."""
    import jax
    import concourse.mybir as mybir_
    from concourse import bass2jax
    from jax.experimental.shard_map import shard_map
    from jax.sharding import Mesh, PartitionSpec

    bass2jax.install_neuronx_cc_hook()
    part_name = (nc.partition_id_tensor.name if nc.partition_id_tensor
                 else None)
    in_names, out_names, out_avals, zero_outs = [], [], [], []
    for alloc in nc.m.functions[0].allocations:
        if not isinstance(alloc, mybir_.MemoryLocationSet):
            continue
        name = alloc.memorylocations[0].name
        if alloc.kind == "ExternalInput":
            if name != part_name:
                in_names.append(name)
        elif alloc.kind == "ExternalOutput":
            shape = tuple(alloc.tensor_shape)
            dtype = mybir_.dt.np(alloc.dtype)
            out_names.append(name)
            out_avals.append(jax.core.ShapedArray(shape, dtype))
            zero_outs.append(np.zeros(shape, dtype))
    n_params = len(in_names)
    all_names = in_names + out_names
    if part_name is not None:
        all_names = all_names + [part_name]
    donate = tuple(range(n_params, n_params + len(out_names)))

    def _body(*args):
        operands = list(args)
        if part_name is not None:
            operands.append(bass2jax.partition_id_tensor())
        outs = bass2jax._bass_exec_p.bind(
            *operands, out_avals=tuple(out_avals), in_names=tuple(all_names),
            out_names=tuple(out_names), lowering_input_output_aliases=(),
            sim_require_finite=True, sim_require_nnan=True, nc=nc)
        return tuple(outs)

    # inputs identical on every core are passed replicated (uploaded once)
    REPL = {"wq_s", "wk_s", "wv_s", "wo_s", "wq_c", "wk_c", "wv_c", "wo_c",
            "w1", "w2", "dmask"}
    in_specs = tuple(PartitionSpec() if nm in REPL else PartitionSpec("core")
                     for nm in in_names) + \
        (PartitionSpec("core"),) * len(out_names)
    devices = jax.devices()[:N_CORES]
    mesh = Mesh(np.asarray(devices), ("core",))
    sharded = jax.jit(
        shard_map(_body, mesh=mesh, in_specs=in_specs,
                  out_specs=(PartitionSpec("core"),) * len(out_names),
                  check_rep=False),
        donate_argnums=donate, keep_unused=True)

    def run(in_maps):
        concat_in = [
            in_maps[0][nm] if nm in REPL else
            np.concatenate([in_maps[c][nm] for c in range(N_CORES)], axis=0)
            for nm in in_names]
        concat_zero = [
            np.zeros((N_CORES * z.shape[0], *z.shape[1:]), z.dtype)
            for z in zero_outs]
        out_arrs = sharded(*concat_in, *concat_zero)
        return [
            {nm: np.asarray(out_arrs[i]).reshape(N_CORES, *out_avals[i].shape)[c]
             for i, nm in enumerate(out_names)}
            for c in range(N_CORES)]

    return run


def kernel(**inputs):
    global _BUILT, _NC
    if _BUILT is None:
        nc = _NC = _build()
        try:
            from concourse._compat import axon_active
            under_axon = axon_active()
        except ImportError:
            under_axon = False
        if under_axon:
            _BUILT = _make_runner(nc)
        else:
            def _native_run(in_maps, _nc=nc):
                res = run_bass_kernel_spmd(_nc, in_maps,
                                           core_ids=list(range(N_CORES)))
                return res.results
            _BUILT = _native_run
    run = _BUILT

    dec = np.asarray(inputs["dec_input"], np.float32)
    enc = np.asarray(inputs["enc_output"], np.float32)
    consts = _prep_consts(inputs)
    in_maps = []
    metas = []
    for cix in range(N_CORES):
        m, meta = _prep_core(cix, dec, enc, consts)
        in_maps.append(m)
        metas.append(meta)

    results = run(in_maps)

    out = np.empty((B, T, D), np.float32)
    for cix in range(N_CORES):
        b, qtok = metas[cix]
        tiles = results[cix]["out"]           # [128, FT, TQ]
        core_t = tiles.transpose(1, 0, 2).reshape(D, TQ)
        out[b, qtok, :] = core_t.T
    return out
